# revision 65
# baseline (speedup 1.0000x reference)
"""Trainium2 Bass kernel for log-softmax multi-head attention (8 NeuronCores).

Reference computation (per batch):
    qkv = x @ w_qkv ; q,k,v per head
    dots = scale * q @ k^T ; attn = log_softmax(dots)
    out = attn @ v  -> merge heads -> out @ w_out + b_out + x

Algebraic identities used:
  1) log_softmax is linear in scores minus a row constant:
       attn = scale*dots - lse,  lse_i = ln sum_j exp(scale*dots_ij)
     so  out_head = scale * q @ (k^T v) - lse (x) colsum(v)
  2) k^T v = Wk^T (x^T x) Wv  (Gram matrix G = x^T x shared by all heads)
  3) colsum(v) = colsum(x) @ Wv
  4) the lse rank-1 correction commutes with the output projection
  so the only O(n^2) work is the score matmul + exp/row-sum pass.

Sharding: 8 cores = 2 batches x 4 query-quarters, outputs disjoint.

Schedule (v2): the exp+rowsum pass is split across ScalarE (exact Exp
ACTIVATE, ~0.95ns/elem + 455ns/instr) and DVE (Schraudolph bit-trick:
i16 = trunc(raw*SC16+SB16) is bf16-bitcast ~exp(SCALE*raw); one 1x pass
from PSUM + a 2x bf16 tensor_tensor halving tree + small cache-reduce).
PSUM = 2 stream slots of [128,1536] (3 banks each) + 2 ride banks, so
score tiles are (1536,1536,1024) pieces per 128-row block.  kT/qT/Gram/
GWk/kv ride on the 2 spare banks during the stream (Gram accumulates
32-matmul chains fully in PSUM); rides keep PE continuously busy which
holds it at the 2.4 GHz p-state (idle PE decays to 1.2 GHz).  Tail does
csx/vsum/W8/Ln + rank-1 correction + output projection as before.
"""

import numpy as np

B, N, D = 2, 4096, 512
H, DH = 8, 64
SCALE = DH**-0.5
NQ = N // 4        # own query rows per core
QT = NQ // 128     # 8 own row tiles
NXT = N // 128     # 32 x row tiles

# Schraudolph fast-exp in bf16 (folding the 1/sqrt(DH) score scale):
#   i16 = trunc(raw * SC16 + SB16); bitcast_bf16(i16) ~ exp(SCALE*raw)
# c16=6.9 zeros the mean relative error (max ~4%/elem, ~0.2% on lse)
SC16 = float(np.float32(SCALE * (2.0**7) / np.log(2.0)))
SB16 = float(np.float32(127.0 * 2.0**7 - 6.9))

# stream piece layout per 128-query-row block: key ranges (1024 first so
# the stream can start once transpose-wave 0 of xT has landed)
PIECES = [(0, 1024), (1024, 2560), (2560, 4096)]

_GRAPH_CACHE = {}


def _build_graph():
    import concourse.bass as bass
    import concourse.tile as tile
    from concourse import bacc, mybir
    from concourse.masks import make_identity

    f32 = mybir.dt.float32
    bf16 = mybir.dt.bfloat16
    i16 = mybir.dt.int16
    AF = mybir.ActivationFunctionType
    ALU = mybir.AluOpType

    nc = bacc.Bacc("TRN2", target_bir_lowering=False, debug=False)

    xbf_d = nc.dram_tensor("x_bf", [N, D], bf16, kind="ExternalInput").ap()
    xq_d = nc.dram_tensor("xq", [NQ, D], f32, kind="ExternalInput").ap()
    wqkv_d = nc.dram_tensor("w_qkv_bf", [D, 3 * D], bf16, kind="ExternalInput").ap()
    wout_d = nc.dram_tensor("w_out_bf", [D, D], bf16, kind="ExternalInput").ap()
    bout_d = nc.dram_tensor("b_out", [D], f32, kind="ExternalInput").ap()
    out_d = nc.dram_tensor("out", [NQ, D], f32, kind="ExternalOutput").ap()

    with tile.TileContext(nc) as tc:
        with (
            tc.tile_pool(name="const", bufs=1) as const,
            tc.tile_pool(name="bigsb", bufs=1) as bigsb,
            tc.tile_pool(name="dout", bufs=2) as dout,
        ):
            # ------- constants + DMAs (issue order = priority) --------------
            ident_bf = const.tile([128, 128], bf16, tag="ident_bf")
            make_identity(nc, ident_bf[:])
            # transposed x wave 0 first (kT0/qT0 gate the stream start)
            xT = [bigsb.tile([128, N], bf16, name=f"xT{j}", tag=f"xT{j}") for j in range(4)]
            for j in range(4):
                nc.sync.dma_start(
                    out=xT[j][:, 0:1024],
                    in_=xbf_d[0:1024, j * 128 : (j + 1) * 128],
                    transpose=True,
                )
            wq = []
            for j in range(4):
                w_t = const.tile([128, 3 * D], bf16, tag=f"wq{j}")
                nc.sync.dma_start(out=w_t[:], in_=wqkv_d[j * 128 : (j + 1) * 128, :])
                wq.append(w_t)
            for r in range(1, 4):
                for j in range(4):
                    nc.sync.dma_start(
                        out=xT[j][:, r * 1024 : (r + 1) * 1024],
                        in_=xbf_d[r * 1024 : (r + 1) * 1024, j * 128 : (j + 1) * 128],
                        transpose=True,
                    )
            # x row tiles (Gram source; gram rides run late in the stream)
            xrow = []
            for t in range(NXT):
                xr_t = bigsb.tile([128, D], bf16, tag=f"xrow{t}")
                nc.sync.dma_start(out=xr_t[:], in_=xbf_d[t * 128 : (t + 1) * 128, :])
                xrow.append(xr_t)
            wo = []
            for j in range(4):
                w_t = const.tile([128, D], bf16, tag=f"wo{j}")
                nc.sync.dma_start(out=w_t[:], in_=wout_d[j * 128 : (j + 1) * 128, :])
                wo.append(w_t)
            b_bc = const.tile([128, D], f32, tag="b_bc")
            nc.sync.dma_start(
                out=b_bc[:],
                in_=bass.AP(
                    tensor=bout_d.tensor,
                    offset=bout_d.offset,
                    ap=[[0, 128]] + [list(p) for p in bout_d.ap],
                ),
            )
            # residual rows (f32) -> become x + b via DVE adds in tail
            xb = []
            for t in range(QT):
                xb_t = dout.tile([128, D], f32, tag=f"xb{t}", bufs=1)
                nc.sync.dma_start(out=xb_t[:], in_=xq_d[t * 128 : (t + 1) * 128, :])
                xb.append(xb_t)

            # ------- big SBUF operands --------------------------------------
            qT = [bigsb.tile([128, NQ], bf16, name=f"qT{c}", tag=f"qT{c}") for c in range(4)]
            kT = [bigsb.tile([128, N], bf16, name=f"kT{c}", tag=f"kT{c}") for c in range(4)]
            G_sb = [bigsb.tile([128, D], f32, name=f"G{j}", tag=f"G{j}") for j in range(4)]
            G_bf = [bigsb.tile([128, D], bf16, name=f"Gb{j}", tag=f"Gb{j}") for j in range(4)]
            GWk = [bigsb.tile([128, D], bf16, name=f"GWk{j}", tag=f"GWk{j}") for j in range(4)]
            KVW = [bigsb.tile([128, D], bf16, name=f"KVW{c}", tag=f"KVW{c}") for c in range(4)]
            kv_p = const.tile([128, D], bf16, tag="kv_p")
            nc.vector.memset(kv_p[:], 0.0)
            csx4 = [const.tile([128, 4], f32, name=f"csx4_{j}", tag=f"csx4_{j}") for j in range(4)]
            csx_bf = [const.tile([128, 1], bf16, name=f"csxb{j}", tag=f"csxb{j}") for j in range(4)]
            vsT = [const.tile([128, 1], bf16, name=f"vsT{j}", tag=f"vsT{j}") for j in range(4)]
            VSmat = [const.tile([128, 8], bf16, name=f"VSm{j}", tag=f"VSm{j}") for j in range(4)]
            for j in range(4):
                nc.vector.memset(VSmat[j][:], 0.0)
            W8_sb = const.tile([8, D], bf16, tag="W8")
            # lse accumulator: col = (h*8+t)*3 + piece
            lse_acc = const.tile([128, 192], f32, tag="lse_acc")
            lse_sum = const.tile([128, 64], f32, tag="lse_sum")
            lse_ln = const.tile([128, 64], bf16, tag="lse_ln")
            lnST = const.tile([8, NQ], bf16, tag="lnST")
            dummy = const.tile([128, 1], f32, tag="dummy")
            nc.vector.memset(dummy[:], 0.0)
            # bit-trick scratch
            scr_i16 = const.tile([128, 1536], i16, tag="scr_i16")
            tr1 = const.tile([128, 768], bf16, tag="tr1")
            tr2 = const.tile([128, 384], bf16, tag="tr2")
            tr3 = const.tile([128, 384], bf16, tag="tr3")

            # preload the Exp table set before the stream
            nc.scalar.activation(out=dummy[:], in_=dummy[:], func=AF.Exp)

            # =================================================================
            # The single PSUM layout for the whole kernel:
            #   P[:, 0:1536]     slot 0 (3 banks)
            #   P[:, 1536:3072]  slot 1 (3 banks)
            #   P[:, 3072:3584]  ride bank A
            #   P[:, 3584:4096]  ride bank B
            # =================================================================
            with tc.tile_pool(name="allps", bufs=1, space="PSUM") as aps:
                P = aps.tile([128, 4096], f32, tag="P")
                slot = [P[:, 0:1536], P[:, 1536:3072]]
                # one ride bank + one filler bank: filler matmuls are
                # zero-dependency PE work that keeps the tensor engine
                # issue stream gap-free so it holds the 2.4 GHz p-state
                rbank = [P[:, 3072:3584], P[:, 3072:3584]]
                fbank = P[:, 3584:4096]

                # ---- ride bodies, split into MM part + lagged drain part ---
                # (drains issued ~2 pieces after their matmuls so they never
                # head-of-line block the consumer engines)
                def kT_mm(c, ch):
                    for j in range(4):
                        nc.tensor.matmul(
                            rbank[0],
                            lhsT=wq[j][:, 512 + c * 128 : 512 + (c + 1) * 128],
                            rhs=xT[j][:, ch * 512 : (ch + 1) * 512],
                            start=(j == 0),
                            stop=(j == 3),
                        )

                def kT_drain(c, ch, d):
                    if d == 0:
                        nc.vector.tensor_copy(kT[c][:, ch * 512 : (ch + 1) * 512], rbank[0])
                    else:
                        nc.scalar.activation(
                            out=kT[c][:, ch * 512 : (ch + 1) * 512], in_=rbank[0],
                            func=AF.Copy,
                        )

                def qT_mm(c, nn):
                    for j in range(4):
                        nc.tensor.matmul(
                            rbank[0],
                            lhsT=wq[j][:, c * 128 : (c + 1) * 128],
                            rhs=xT[j][:, nn * 512 : (nn + 1) * 512],
                            start=(j == 0),
                            stop=(j == 3),
                        )

                def qT_drain(c, nn, d):
                    if d == 0:
                        nc.vector.tensor_copy(qT[c][:, nn * 512 : (nn + 1) * 512], rbank[0])
                    else:
                        nc.scalar.activation(
                            out=qT[c][:, nn * 512 : (nn + 1) * 512], in_=rbank[0],
                            func=AF.Copy,
                        )

                def gram_mm(jm, t0):
                    for t in range(t0, t0 + 16):
                        nc.tensor.matmul(
                            rbank[0],
                            lhsT=xrow[t][:, jm * 128 : (jm + 1) * 128],
                            rhs=xrow[t][:],
                            start=(t == t0),
                            stop=(t == t0 + 15),
                        )

                def gram_drain(jm, t0):
                    if t0 == 0:
                        nc.vector.tensor_copy(G_sb[jm][:], rbank[0])
                    else:
                        nc.vector.tensor_add(G_sb[jm][:], G_sb[jm][:], rbank[0])
                        nc.vector.tensor_copy(G_bf[jm][:], G_sb[jm][:])

                def gwk_mm(jm):
                    for j in range(4):
                        nc.tensor.matmul(
                            rbank[0],
                            lhsT=G_bf[j][:, jm * 128 : (jm + 1) * 128],
                            rhs=wq[j][:, 512:1024],
                            start=(j == 0),
                            stop=(j == 3),
                        )

                def gwk_drain(jm, d):
                    if d == 0:
                        nc.vector.tensor_copy(GWk[jm][:], rbank[0])
                    else:
                        nc.scalar.activation(out=GWk[jm][:], in_=rbank[0], func=AF.Copy)

                def kvt_mm(h):
                    for j in range(4):
                        nc.tensor.matmul(
                            rbank[0][0:64, 0:64],
                            lhsT=wq[j][:, 1024 + h * 64 : 1024 + (h + 1) * 64],
                            rhs=GWk[j][:, h * 64 : (h + 1) * 64],
                            start=(j == 0),
                            stop=(j == 3),
                        )

                def kvt_drain(h):
                    r0 = (h % 2) * 64
                    nc.vector.tensor_scalar_mul(
                        kv_p[r0 : r0 + 64, h * 64 : (h + 1) * 64],
                        rbank[0][0:64, 0:64], SCALE,
                    )

                # PE warm-up: zero-dep matmuls run during the DMA wait so
                # the head and first stream pieces execute at 2.4 GHz
                for _ in range(16):
                    nc.tensor.matmul(
                        fbank[:, 0:128], lhsT=ident_bf[:], rhs=ident_bf[:],
                        start=True, stop=True,
                    )

                # ---- head: kT0 ch0-1 + qT0 (xT wave 0 + wq only) -----------
                kT_mm(0, 0)
                kT_drain(0, 0, 0)
                kT_mm(0, 1)
                kT_drain(0, 1, 1)
                qT_mm(0, 0)
                qT_drain(0, 0, 0)
                qT_mm(0, 1)
                qT_drain(0, 1, 1)

                # ---- ride schedule: (target piece, mm_fn, drain_fn) --------
                # single ride bank; ordered by deadline (kT[c] first needed
                # at piece 48c; gram/gwk/kv only by the tail)
                rides = []
                for i, ch in enumerate(range(2, 8)):         # kT0 tail
                    rides.append((2 * (i + 1),
                                  lambda ch=ch: kT_mm(0, ch),
                                  lambda ch=ch, d=ch % 2: kT_drain(0, ch, d)))
                for c in range(1, 4):                        # kT1-3 + qT1-3
                    base = 36 * c - 20
                    for ch in range(8):
                        rides.append((base + 3 * ch,
                                      lambda c=c, ch=ch: kT_mm(c, ch),
                                      lambda c=c, ch=ch, d=(ch + c) % 2: kT_drain(c, ch, d)))
                    rides.append((base + 24, lambda c=c: qT_mm(c, 0),
                                  lambda c=c: qT_drain(c, 0, 0)))
                    rides.append((base + 27, lambda c=c: qT_mm(c, 1),
                                  lambda c=c: qT_drain(c, 1, 1)))
                for jm in range(4):                          # Gram halves
                    for s16 in range(2):
                        rides.append((112 + 6 * jm + 3 * s16,
                                      lambda jm=jm, t0=16 * s16: gram_mm(jm, t0),
                                      lambda jm=jm, t0=16 * s16: gram_drain(jm, t0)))
                for jm in range(4):                          # G @ Wk
                    rides.append((140 + 3 * jm, lambda jm=jm: gwk_mm(jm),
                                  lambda jm=jm, d=jm % 2: gwk_drain(jm, d)))
                for h in range(H):                           # kv per head
                    rides.append((152 + 2 * h, lambda h=h: kvt_mm(h),
                                  lambda h=h: kvt_drain(h)))

                # late-stream DVE/PE rides: csx, xb, vsum, W8, KVW (the only
                # true tail is the Ln -> rank-1 -> projection chain)
                def csx_piece(j, p):
                    nc.vector.tensor_reduce(
                        csx4[j][:, p : p + 1],
                        xT[j][:, p * 1024 : (p + 1) * 1024],
                        axis=mybir.AxisListType.X,
                        op=ALU.add,
                    )

                def csx_fin(j):
                    nc.vector.tensor_reduce(
                        csx4[j][:, 0:1], csx4[j][:],
                        axis=mybir.AxisListType.X, op=ALU.add,
                    )
                    nc.vector.tensor_copy(csx_bf[j][:], csx4[j][:, 0:1])

                def xb_add(t):
                    nc.vector.tensor_add(xb[t][:], xb[t][:], b_bc[:])

                def vsum_mm(jm):
                    for j in range(4):
                        nc.tensor.matmul(
                            rbank[0][:, 0:1],
                            lhsT=wq[j][:, 1024 + jm * 128 : 1024 + (jm + 1) * 128],
                            rhs=csx_bf[j][:],
                            start=(j == 0),
                            stop=(j == 3),
                        )

                def vsum_drain(jm):
                    nc.vector.tensor_scalar_mul(vsT[jm][:], rbank[0][:, 0:1], -1.0)
                    nc.vector.tensor_copy(
                        VSmat[jm][0:64, 2 * jm : 2 * jm + 1], vsT[jm][0:64, :]
                    )
                    nc.vector.tensor_copy(
                        VSmat[jm][64:128, 2 * jm + 1 : 2 * jm + 2], vsT[jm][64:128, :]
                    )

                def w8_mm():
                    for j in range(4):
                        nc.tensor.matmul(
                            rbank[0][0:8, :],
                            lhsT=VSmat[j][:],
                            rhs=wo[j][:],
                            start=(j == 0),
                            stop=(j == 3),
                        )

                def w8_drain():
                    nc.vector.tensor_copy(W8_sb[:], rbank[0][0:8, :])

                def kvw_mm(c):
                    for hp in range(2):
                        h, r0h = 2 * c + hp, hp * 64
                        nc.tensor.matmul(
                            rbank[0][r0h : r0h + 64, :],
                            lhsT=kv_p[:, h * 64 : (h + 1) * 64],
                            rhs=wo[c][:],
                            start=True,
                            stop=True,
                        )

                def kvw_drain(c):
                    nc.vector.tensor_copy(KVW[c][:], rbank[0])

                for i in range(16):
                    rides.append((118 + 2 * i, None,
                                  lambda j=i // 4, p=i % 4: csx_piece(j, p)))
                for j in range(4):
                    rides.append((151 + j, None, lambda j=j: csx_fin(j)))
                for t in range(QT):
                    rides.append((139 + 2 * t, None, lambda t=t: xb_add(t)))
                for jm in range(4):
                    rides.append((155 + 3 * jm, lambda jm=jm: vsum_mm(jm),
                                  lambda jm=jm: vsum_drain(jm)))
                rides.append((183, w8_mm, w8_drain))
                for c in range(4):
                    rides.append((168 + 3 * c, lambda c=c: kvw_mm(c),
                                  lambda c=c: kvw_drain(c)))
                rides.sort(key=lambda r: r[0])

                # ---- the exp stream: (h, piece, t); rides + filler paced --
                # DVE takes DVE_SH of 192 pieces; filler matmuls keep the PE
                # issue stream from idling (p-state)
                DVE_SH = 60
                LAG = 0
                state = {"ride": 0}
                drainq = []

                def filler_mm():
                    nc.tensor.matmul(
                        fbank, lhsT=wq[0][:, 0:128], rhs=wq[0][:, 0:512],
                        start=True, stop=True,
                    )

                def pace(pidx, dots_cols):
                    # drains from rides issued >= LAG pieces ago: their
                    # matmuls are long retired, so the consumer-engine copy
                    # dispatches without stalling the ops behind it
                    while drainq and pidx >= drainq[0][0]:
                        drainq.pop(0)[1]()
                    nmm = 0
                    while state["ride"] < len(rides) and pidx >= rides[state["ride"]][0]:
                        tgt, mm_fn, drain_fn = rides[state["ride"]]
                        if mm_fn is not None:
                            # program order must keep the pending drain
                            # before the next bank write
                            if drainq:
                                break
                            mm_fn()
                            nmm = 1
                            if drain_fn is not None:
                                drainq.append((pidx + LAG, drain_fn))
                        elif drain_fn is not None:
                            drain_fn()
                        state["ride"] += 1
                        if nmm:
                            break  # one bank ride per piece max
                    filler_mm()
                    if not nmm:
                        filler_mm()

                pidx = 0
                # group order defers each head-pair's high-key piece 2 until
                # after both heads' pieces 0-1, giving the wave-2/3 transpose
                # DMAs and kT ch5-7 rides ~15 extra pieces of slack (the
                # early stream otherwise stalls on them)
                order = []
                for cc_ in range(4):
                    h0, h1 = 2 * cc_, 2 * cc_ + 1
                    order += [(h0, 0), (h0, 1), (h1, 0), (h1, 1),
                              (h0, 2), (h1, 2)]
                for h, piece in order:
                    c, r0 = h // 2, (h % 2) * 64
                    k0, k1 = PIECES[piece]
                    if True:
                        for t in range(QT):
                            s = slot[pidx % 2]
                            fd = k1 - k0
                            lhsT = qT[c][r0 : r0 + 64, t * 128 : (t + 1) * 128]
                            for cc in range((fd + 511) // 512):
                                nc.tensor.matmul(
                                    s[:, cc * 512 : min((cc + 1) * 512, fd)],
                                    lhsT=lhsT,
                                    rhs=kT[c][r0 : r0 + 64, k0 + cc * 512 : min(k0 + (cc + 1) * 512, k1)],
                                    start=True,
                                    stop=True,
                                )
                            pace(pidx, fd)
                            col = (h * 8 + t) * 3 + piece
                            if (pidx * DVE_SH) // 192 != ((pidx + 1) * DVE_SH) // 192:
                                # DVE bit-trick path
                                nc.vector.tensor_scalar(
                                    out=scr_i16[:, 0:fd],
                                    in0=s[:, 0:fd],
                                    scalar1=SC16,
                                    scalar2=SB16,
                                    op0=ALU.mult,
                                    op1=ALU.add,
                                )
                                hf = fd // 2
                                nc.vector.tensor_tensor(
                                    out=tr1[:, 0 : hf // 1],
                                    in0=scr_i16[:, 0:hf].bitcast(bf16),
                                    in1=scr_i16[:, hf:fd].bitcast(bf16),
                                    op=ALU.add,
                                )
                                qf = hf // 2
                                nc.vector.tensor_tensor(
                                    out=tr2[:, 0:qf],
                                    in0=tr1[:, 0:qf],
                                    in1=tr1[:, qf:hf],
                                    op=ALU.add,
                                )
                                nc.vector.tensor_scalar(
                                    out=tr3[:, 0:qf],
                                    in0=tr2[:, 0:qf],
                                    scalar1=1.0,
                                    scalar2=None,
                                    op0=ALU.mult,
                                    op1=ALU.add,
                                    accum_out=lse_acc[:, col : col + 1],
                                )
                            else:
                                nc.scalar.activation(
                                    out=s[:, 0:fd],
                                    in_=s[:, 0:fd],
                                    func=AF.Exp,
                                    scale=SCALE,
                                    accum_out=lse_acc[:, col : col + 1],
                                )
                            pidx += 1
                # flush leftover rides + drains (post-stream)
                while drainq:
                    drainq.pop(0)[1]()
                while state["ride"] < len(rides):
                    _, mm_fn, drain_fn = rides[state["ride"]]
                    if mm_fn is not None:
                        mm_fn()
                    if drain_fn is not None:
                        drain_fn()
                    state["ride"] += 1

                # ---- tail: Ln + rank-1 + projection ------------------------
                # lse: sum the 3 per-piece accumulator cols -> [128, 64]
                la = lse_acc[:].rearrange("q (p three) -> q p three", three=3)
                nc.vector.tensor_add(lse_sum[:], la[:, :, 0], la[:, :, 1])
                nc.vector.tensor_add(lse_sum[:], lse_sum[:], la[:, :, 2])
                nc.scalar.activation(out=lse_ln[:], in_=lse_sum[:], func=AF.Ln)
                lse_tm = const.tile([128, 64], bf16, tag="lse_tm")
                nc.vector.tensor_copy(
                    lse_tm[:],
                    lse_ln[:].rearrange("q (h t) -> q t h", t=QT),
                )
                for t in range(QT):
                    sl = slot[t % 2]
                    ps_bf = sl[0:8, 0:64].bitcast(bf16)
                    nc.tensor.transpose(ps_bf, lse_tm[:, t * 8 : (t + 1) * 8], ident_bf[:])
                    nc.vector.tensor_copy(lnST[:, t * 128 : (t + 1) * 128], ps_bf)
                    yps = sl[:, 512:1024]
                    for c in range(4):
                        nc.tensor.matmul(
                            yps,
                            lhsT=qT[c][:, t * 128 : (t + 1) * 128],
                            rhs=KVW[c][:],
                            start=(c == 0),
                            stop=False,
                        )
                    nc.tensor.matmul(
                        yps,
                        lhsT=lnST[:, t * 128 : (t + 1) * 128],
                        rhs=W8_sb[:],
                        start=False,
                        stop=True,
                    )
                    ysb = dout.tile([128, D], f32, name="ysb", tag="ysb")
                    nc.vector.tensor_add(ysb[:], yps, xb[t][:])
                    nc.sync.dma_start(out=out_d[t * 128 : (t + 1) * 128, :], in_=ysb[:])

    nc.compile()
    return nc


def get_graph():
    if "nc" not in _GRAPH_CACHE:
        _GRAPH_CACHE["nc"] = _build_graph()
    return _GRAPH_CACHE["nc"]


def make_in_maps(x, w_qkv, w_out, b_out):
    import ml_dtypes

    x = np.ascontiguousarray(x, dtype=np.float32)
    w_qkv = np.ascontiguousarray(w_qkv, dtype=np.float32)
    w_out = np.ascontiguousarray(w_out, dtype=np.float32)
    b_out = np.ascontiguousarray(b_out, dtype=np.float32)
    x_bf = x.astype(ml_dtypes.bfloat16)
    w_qkv_bf = w_qkv.astype(ml_dtypes.bfloat16)
    w_out_bf = w_out.astype(ml_dtypes.bfloat16)
    in_maps = []
    for i in range(8):
        b, q = divmod(i, 4)
        in_maps.append(
            {
                # keys are permutation-invariant for lse/kv/G; roll so this
                # core's own query rows sit at rows 0:NQ
                "x_bf": np.ascontiguousarray(np.roll(x_bf[b], -q * NQ, axis=0)),
                "xq": np.ascontiguousarray(x[b, q * NQ : (q + 1) * NQ]),
                "w_qkv_bf": w_qkv_bf,
                "w_out_bf": w_out_bf,
                "b_out": b_out,
            }
        )
    return in_maps


def kernel(x, w_qkv, w_out, b_out):
    from concourse.bass_utils import run_bass_kernel_spmd

    nc = get_graph()
    in_maps = make_in_maps(x, w_qkv, w_out, b_out)
    res = run_bass_kernel_spmd(nc, in_maps, core_ids=list(range(8)))
    out = np.empty((B, N, D), np.float32)
    for i in range(8):
        b, q = divmod(i, 4)
        out[b, q * NQ : (q + 1) * NQ] = res.results[i]["out"]
    return out


# revision 66
# speedup vs baseline: 1.0345x; 1.0345x over previous
"""Trainium2 Bass kernel for log-softmax multi-head attention (8 NeuronCores).

Reference computation (per batch):
    qkv = x @ w_qkv ; q,k,v per head
    dots = scale * q @ k^T ; attn = log_softmax(dots)
    out = attn @ v  -> merge heads -> out @ w_out + b_out + x

Algebraic identities used:
  1) log_softmax is linear in scores minus a row constant:
       attn = scale*dots - lse,  lse_i = ln sum_j exp(scale*dots_ij)
     so  out_head = scale * q @ (k^T v) - lse (x) colsum(v)
  2) k^T v = Wk^T (x^T x) Wv  (Gram matrix G = x^T x shared by all heads)
  3) colsum(v) = colsum(x) @ Wv
  4) the lse rank-1 correction commutes with the output projection
  so the only O(n^2) work is the score matmul + exp/row-sum pass.

Sharding: 8 cores = 2 batches x 4 query-quarters, outputs disjoint.

Schedule (v2): the exp+rowsum pass is split across ScalarE (exact Exp
ACTIVATE, ~0.95ns/elem + 455ns/instr) and DVE (Schraudolph bit-trick:
i16 = trunc(raw*SC16+SB16) is bf16-bitcast ~exp(SCALE*raw); one 1x pass
from PSUM + a 2x bf16 tensor_tensor halving tree + small cache-reduce).
PSUM = 2 stream slots of [128,1536] (3 banks each) + 2 ride banks, so
score tiles are (1536,1536,1024) pieces per 128-row block.  kT/qT/Gram/
GWk/kv ride on the 2 spare banks during the stream (Gram accumulates
32-matmul chains fully in PSUM); rides keep PE continuously busy which
holds it at the 2.4 GHz p-state (idle PE decays to 1.2 GHz).  Tail does
csx/vsum/W8/Ln + rank-1 correction + output projection as before.
"""

import numpy as np

B, N, D = 2, 4096, 512
H, DH = 8, 64
SCALE = DH**-0.5
NQ = N // 4        # own query rows per core
QT = NQ // 128     # 8 own row tiles
NXT = N // 128     # 32 x row tiles

# Schraudolph fast-exp in bf16 (folding the 1/sqrt(DH) score scale):
#   i16 = trunc(raw * SC16 + SB16); bitcast_bf16(i16) ~ exp(SCALE*raw)
# c16=6.9 zeros the mean relative error (max ~4%/elem, ~0.2% on lse)
SC16 = float(np.float32(SCALE * (2.0**7) / np.log(2.0)))
SB16 = float(np.float32(127.0 * 2.0**7 - 6.9))

# stream piece layout per 128-query-row block: key ranges (1024 first so
# the stream can start once transpose-wave 0 of xT has landed)
PIECES = [(0, 1024), (1024, 2560), (2560, 4096)]

_GRAPH_CACHE = {}


def _build_graph():
    import concourse.bass as bass
    import concourse.tile as tile
    from concourse import bacc, mybir
    from concourse.masks import make_identity

    f32 = mybir.dt.float32
    bf16 = mybir.dt.bfloat16
    i16 = mybir.dt.int16
    AF = mybir.ActivationFunctionType
    ALU = mybir.AluOpType

    nc = bacc.Bacc("TRN2", target_bir_lowering=False, debug=False)

    xbf_d = nc.dram_tensor("x_bf", [N, D], bf16, kind="ExternalInput").ap()
    xq_d = nc.dram_tensor("xq", [NQ, D], f32, kind="ExternalInput").ap()
    wqkv_d = nc.dram_tensor("w_qkv_bf", [D, 3 * D], bf16, kind="ExternalInput").ap()
    wout_d = nc.dram_tensor("w_out_bf", [D, D], bf16, kind="ExternalInput").ap()
    bout_d = nc.dram_tensor("b_out", [D], f32, kind="ExternalInput").ap()
    out_d = nc.dram_tensor("out", [NQ, D], f32, kind="ExternalOutput").ap()

    with tile.TileContext(nc) as tc:
        with (
            tc.tile_pool(name="const", bufs=1) as const,
            tc.tile_pool(name="bigsb", bufs=1) as bigsb,
            tc.tile_pool(name="dout", bufs=2) as dout,
        ):
            # ------- constants + DMAs (issue order = priority) --------------
            ident_bf = const.tile([128, 128], bf16, tag="ident_bf")
            make_identity(nc, ident_bf[:])
            wq = []
            for j in range(4):
                w_t = const.tile([128, 3 * D], bf16, tag=f"wq{j}")
                nc.sync.dma_start(out=w_t[:], in_=wqkv_d[j * 128 : (j + 1) * 128, :])
                wq.append(w_t)
            # transposed x (kT/qT source): halves 0:2048 first -> kT0/qT0;
            # merged [128,2048] transpose DMAs (Sync issue cost ~1.3us each)
            xT = [bigsb.tile([128, N], bf16, name=f"xT{j}", tag=f"xT{j}") for j in range(4)]
            for r in range(4):
                for j in range(4):
                    nc.sync.dma_start(
                        out=xT[j][:, r * 1024 : (r + 1) * 1024],
                        in_=xbf_d[r * 1024 : (r + 1) * 1024, j * 128 : (j + 1) * 128],
                        transpose=True,
                    )
            # x row tiles (Gram source; gram rides run late in the stream)
            xrow = []
            for t in range(NXT):
                xr_t = bigsb.tile([128, D], bf16, tag=f"xrow{t}")
                nc.sync.dma_start(out=xr_t[:], in_=xbf_d[t * 128 : (t + 1) * 128, :])
                xrow.append(xr_t)
            wo = []
            for j in range(4):
                w_t = const.tile([128, D], bf16, tag=f"wo{j}")
                nc.sync.dma_start(out=w_t[:], in_=wout_d[j * 128 : (j + 1) * 128, :])
                wo.append(w_t)
            b_bc = const.tile([128, D], f32, tag="b_bc")
            nc.sync.dma_start(
                out=b_bc[:],
                in_=bass.AP(
                    tensor=bout_d.tensor,
                    offset=bout_d.offset,
                    ap=[[0, 128]] + [list(p) for p in bout_d.ap],
                ),
            )
            # residual rows (f32) -> become x + b via DVE adds in tail
            xb = []
            for t in range(QT):
                xb_t = dout.tile([128, D], f32, tag=f"xb{t}", bufs=1)
                nc.sync.dma_start(out=xb_t[:], in_=xq_d[t * 128 : (t + 1) * 128, :])
                xb.append(xb_t)

            # ------- big SBUF operands --------------------------------------
            qT = [bigsb.tile([128, NQ], bf16, name=f"qT{c}", tag=f"qT{c}") for c in range(4)]
            kT = [bigsb.tile([128, N], bf16, name=f"kT{c}", tag=f"kT{c}") for c in range(4)]
            G_sb = [bigsb.tile([128, D], f32, name=f"G{j}", tag=f"G{j}") for j in range(4)]
            G_bf = [bigsb.tile([128, D], bf16, name=f"Gb{j}", tag=f"Gb{j}") for j in range(4)]
            GWk = [bigsb.tile([128, D], bf16, name=f"GWk{j}", tag=f"GWk{j}") for j in range(4)]
            KVW = [bigsb.tile([128, D], bf16, name=f"KVW{c}", tag=f"KVW{c}") for c in range(4)]
            kv_p = const.tile([128, D], bf16, tag="kv_p")
            nc.vector.memset(kv_p[:], 0.0)
            csx4 = [const.tile([128, 4], f32, name=f"csx4_{j}", tag=f"csx4_{j}") for j in range(4)]
            csx_bf = [const.tile([128, 1], bf16, name=f"csxb{j}", tag=f"csxb{j}") for j in range(4)]
            vsT = [const.tile([128, 1], bf16, name=f"vsT{j}", tag=f"vsT{j}") for j in range(4)]
            VSmat = [const.tile([128, 8], bf16, name=f"VSm{j}", tag=f"VSm{j}") for j in range(4)]
            for j in range(4):
                nc.vector.memset(VSmat[j][:], 0.0)
            W8_sb = const.tile([8, D], bf16, tag="W8")
            # lse accumulator: col = (h*8+t)*3 + piece
            lse_acc = const.tile([128, 192], f32, tag="lse_acc")
            lse_sum = const.tile([128, 64], f32, tag="lse_sum")
            lse_ln = const.tile([128, 64], bf16, tag="lse_ln")
            lnST = const.tile([8, NQ], bf16, tag="lnST")
            dummy = const.tile([128, 1], f32, tag="dummy")
            nc.vector.memset(dummy[:], 0.0)
            # bit-trick scratch
            scr_i16 = const.tile([128, 1536], i16, tag="scr_i16")
            tr1 = const.tile([128, 768], bf16, tag="tr1")
            tr2 = const.tile([128, 384], bf16, tag="tr2")
            tr3 = const.tile([128, 384], bf16, tag="tr3")

            # preload the Exp table set before the stream
            nc.scalar.activation(out=dummy[:], in_=dummy[:], func=AF.Exp)

            # =================================================================
            # The single PSUM layout for the whole kernel:
            #   P[:, 0:1536]     slot 0 (3 banks)
            #   P[:, 1536:3072]  slot 1 (3 banks)
            #   P[:, 3072:3584]  ride bank A
            #   P[:, 3584:4096]  ride bank B
            # =================================================================
            with tc.tile_pool(name="allps", bufs=1, space="PSUM") as aps:
                P = aps.tile([128, 4096], f32, tag="P")
                slot = [P[:, 0:1536], P[:, 1536:3072]]
                # one ride bank + one filler bank: filler matmuls are
                # zero-dependency PE work that keeps the tensor engine
                # issue stream gap-free so it holds the 2.4 GHz p-state
                rbank = [P[:, 3072:3584], P[:, 3072:3584]]
                fbank = P[:, 3584:4096]

                # ---- ride bodies, split into MM part + lagged drain part ---
                # (drains issued ~2 pieces after their matmuls so they never
                # head-of-line block the consumer engines)
                def kT_mm(c, ch):
                    for j in range(4):
                        nc.tensor.matmul(
                            rbank[0],
                            lhsT=wq[j][:, 512 + c * 128 : 512 + (c + 1) * 128],
                            rhs=xT[j][:, ch * 512 : (ch + 1) * 512],
                            start=(j == 0),
                            stop=(j == 3),
                        )

                def kT_drain(c, ch, d):
                    if d == 0:
                        nc.vector.tensor_copy(kT[c][:, ch * 512 : (ch + 1) * 512], rbank[0])
                    else:
                        nc.scalar.activation(
                            out=kT[c][:, ch * 512 : (ch + 1) * 512], in_=rbank[0],
                            func=AF.Copy,
                        )

                def qT_mm(c, nn):
                    for j in range(4):
                        nc.tensor.matmul(
                            rbank[0],
                            lhsT=wq[j][:, c * 128 : (c + 1) * 128],
                            rhs=xT[j][:, nn * 512 : (nn + 1) * 512],
                            start=(j == 0),
                            stop=(j == 3),
                        )

                def qT_drain(c, nn, d):
                    if d == 0:
                        nc.vector.tensor_copy(qT[c][:, nn * 512 : (nn + 1) * 512], rbank[0])
                    else:
                        nc.scalar.activation(
                            out=qT[c][:, nn * 512 : (nn + 1) * 512], in_=rbank[0],
                            func=AF.Copy,
                        )

                def gram_mm(jm, t0):
                    for t in range(t0, t0 + 16):
                        nc.tensor.matmul(
                            rbank[0],
                            lhsT=xrow[t][:, jm * 128 : (jm + 1) * 128],
                            rhs=xrow[t][:],
                            start=(t == t0),
                            stop=(t == t0 + 15),
                        )

                def gram_drain(jm, t0):
                    if t0 == 0:
                        nc.vector.tensor_copy(G_sb[jm][:], rbank[0])
                    else:
                        nc.vector.tensor_add(G_sb[jm][:], G_sb[jm][:], rbank[0])
                        nc.vector.tensor_copy(G_bf[jm][:], G_sb[jm][:])

                def gwk_mm(jm):
                    for j in range(4):
                        nc.tensor.matmul(
                            rbank[0],
                            lhsT=G_bf[j][:, jm * 128 : (jm + 1) * 128],
                            rhs=wq[j][:, 512:1024],
                            start=(j == 0),
                            stop=(j == 3),
                        )

                def gwk_drain(jm, d):
                    if d == 0:
                        nc.vector.tensor_copy(GWk[jm][:], rbank[0])
                    else:
                        nc.scalar.activation(out=GWk[jm][:], in_=rbank[0], func=AF.Copy)

                def kvt_mm(h):
                    for j in range(4):
                        nc.tensor.matmul(
                            rbank[0][0:64, 0:64],
                            lhsT=wq[j][:, 1024 + h * 64 : 1024 + (h + 1) * 64],
                            rhs=GWk[j][:, h * 64 : (h + 1) * 64],
                            start=(j == 0),
                            stop=(j == 3),
                        )

                def kvt_drain(h):
                    r0 = (h % 2) * 64
                    nc.vector.tensor_scalar_mul(
                        kv_p[r0 : r0 + 64, h * 64 : (h + 1) * 64],
                        rbank[0][0:64, 0:64], SCALE,
                    )

                # ---- head: kT0 ch0-1 + qT0 (xT wave 0 + wq only) -----------
                kT_mm(0, 0)
                kT_drain(0, 0, 0)
                kT_mm(0, 1)
                kT_drain(0, 1, 1)
                qT_mm(0, 0)
                qT_drain(0, 0, 0)
                qT_mm(0, 1)
                qT_drain(0, 1, 1)

                # ---- ride schedule: (target piece, mm_fn, drain_fn) --------
                # single ride bank; ordered by deadline (kT[c] first needed
                # at piece 48c; gram/gwk/kv only by the tail)
                rides = []
                for i, ch in enumerate(range(2, 8)):         # kT0 tail
                    rides.append((2 * (i + 1),
                                  lambda ch=ch: kT_mm(0, ch),
                                  lambda ch=ch, d=ch % 2: kT_drain(0, ch, d)))
                for c in range(1, 4):                        # kT1-3 + qT1-3
                    base = 36 * c - 20
                    for ch in range(8):
                        rides.append((base + 3 * ch,
                                      lambda c=c, ch=ch: kT_mm(c, ch),
                                      lambda c=c, ch=ch, d=(ch + c) % 2: kT_drain(c, ch, d)))
                    rides.append((base + 24, lambda c=c: qT_mm(c, 0),
                                  lambda c=c: qT_drain(c, 0, 0)))
                    rides.append((base + 27, lambda c=c: qT_mm(c, 1),
                                  lambda c=c: qT_drain(c, 1, 1)))
                for jm in range(4):                          # Gram halves
                    for s16 in range(2):
                        rides.append((112 + 6 * jm + 3 * s16,
                                      lambda jm=jm, t0=16 * s16: gram_mm(jm, t0),
                                      lambda jm=jm, t0=16 * s16: gram_drain(jm, t0)))
                for jm in range(4):                          # G @ Wk
                    rides.append((140 + 3 * jm, lambda jm=jm: gwk_mm(jm),
                                  lambda jm=jm, d=jm % 2: gwk_drain(jm, d)))
                for h in range(H):                           # kv per head
                    rides.append((152 + 2 * h, lambda h=h: kvt_mm(h),
                                  lambda h=h: kvt_drain(h)))

                # late-stream DVE/PE rides: csx, xb, vsum, W8, KVW (the only
                # true tail is the Ln -> rank-1 -> projection chain)
                def csx_piece(j, p):
                    nc.vector.tensor_reduce(
                        csx4[j][:, p : p + 1],
                        xT[j][:, p * 1024 : (p + 1) * 1024],
                        axis=mybir.AxisListType.X,
                        op=ALU.add,
                    )

                def csx_fin(j):
                    nc.vector.tensor_reduce(
                        csx4[j][:, 0:1], csx4[j][:],
                        axis=mybir.AxisListType.X, op=ALU.add,
                    )
                    nc.vector.tensor_copy(csx_bf[j][:], csx4[j][:, 0:1])

                def xb_add(t):
                    nc.vector.tensor_add(xb[t][:], xb[t][:], b_bc[:])

                def vsum_mm(jm):
                    for j in range(4):
                        nc.tensor.matmul(
                            rbank[0][:, 0:1],
                            lhsT=wq[j][:, 1024 + jm * 128 : 1024 + (jm + 1) * 128],
                            rhs=csx_bf[j][:],
                            start=(j == 0),
                            stop=(j == 3),
                        )

                def vsum_drain(jm):
                    nc.vector.tensor_scalar_mul(vsT[jm][:], rbank[0][:, 0:1], -1.0)
                    nc.vector.tensor_copy(
                        VSmat[jm][0:64, 2 * jm : 2 * jm + 1], vsT[jm][0:64, :]
                    )
                    nc.vector.tensor_copy(
                        VSmat[jm][64:128, 2 * jm + 1 : 2 * jm + 2], vsT[jm][64:128, :]
                    )

                def w8_mm():
                    for j in range(4):
                        nc.tensor.matmul(
                            rbank[0][0:8, :],
                            lhsT=VSmat[j][:],
                            rhs=wo[j][:],
                            start=(j == 0),
                            stop=(j == 3),
                        )

                def w8_drain():
                    nc.vector.tensor_copy(W8_sb[:], rbank[0][0:8, :])

                def kvw_mm(c):
                    for hp in range(2):
                        h, r0h = 2 * c + hp, hp * 64
                        nc.tensor.matmul(
                            rbank[0][r0h : r0h + 64, :],
                            lhsT=kv_p[:, h * 64 : (h + 1) * 64],
                            rhs=wo[c][:],
                            start=True,
                            stop=True,
                        )

                def kvw_drain(c):
                    nc.vector.tensor_copy(KVW[c][:], rbank[0])

                for i in range(16):
                    rides.append((118 + 2 * i, None,
                                  lambda j=i // 4, p=i % 4: csx_piece(j, p)))
                for j in range(4):
                    rides.append((151 + j, None, lambda j=j: csx_fin(j)))
                for t in range(QT):
                    rides.append((139 + 2 * t, None, lambda t=t: xb_add(t)))
                for jm in range(4):
                    rides.append((155 + 3 * jm, lambda jm=jm: vsum_mm(jm),
                                  lambda jm=jm: vsum_drain(jm)))
                rides.append((183, w8_mm, w8_drain))
                for c in range(4):
                    rides.append((168 + 3 * c, lambda c=c: kvw_mm(c),
                                  lambda c=c: kvw_drain(c)))
                rides.sort(key=lambda r: r[0])

                # ---- the exp stream: (h, piece, t); rides + filler paced --
                # DVE takes DVE_SH of 192 pieces; filler matmuls keep the PE
                # issue stream from idling (p-state)
                DVE_SH = 60
                LAG = 0
                state = {"ride": 0}
                drainq = []

                def filler_mm():
                    nc.tensor.matmul(
                        fbank, lhsT=wq[0][:, 0:128], rhs=wq[0][:, 0:512],
                        start=True, stop=True,
                    )

                def pace(pidx, dots_cols):
                    # drains from rides issued >= LAG pieces ago: their
                    # matmuls are long retired, so the consumer-engine copy
                    # dispatches without stalling the ops behind it
                    while drainq and pidx >= drainq[0][0]:
                        drainq.pop(0)[1]()
                    nmm = 0
                    while state["ride"] < len(rides) and pidx >= rides[state["ride"]][0]:
                        tgt, mm_fn, drain_fn = rides[state["ride"]]
                        if mm_fn is not None:
                            # program order must keep the pending drain
                            # before the next bank write
                            if drainq:
                                break
                            mm_fn()
                            nmm = 1
                            if drain_fn is not None:
                                drainq.append((pidx + LAG, drain_fn))
                        elif drain_fn is not None:
                            drain_fn()
                        state["ride"] += 1
                        if nmm:
                            break  # one bank ride per piece max
                    filler_mm()
                    if not nmm:
                        filler_mm()

                pidx = 0
                # group order defers each head-pair's high-key piece 2 until
                # after both heads' pieces 0-1, giving the wave-2/3 transpose
                # DMAs and kT ch5-7 rides ~15 extra pieces of slack (the
                # early stream otherwise stalls on them)
                order = []
                for cc_ in range(4):
                    h0, h1 = 2 * cc_, 2 * cc_ + 1
                    order += [(h0, 0), (h0, 1), (h1, 0), (h1, 1),
                              (h0, 2), (h1, 2)]
                for h, piece in order:
                    c, r0 = h // 2, (h % 2) * 64
                    k0, k1 = PIECES[piece]
                    if True:
                        for t in range(QT):
                            s = slot[pidx % 2]
                            fd = k1 - k0
                            lhsT = qT[c][r0 : r0 + 64, t * 128 : (t + 1) * 128]
                            for cc in range((fd + 511) // 512):
                                nc.tensor.matmul(
                                    s[:, cc * 512 : min((cc + 1) * 512, fd)],
                                    lhsT=lhsT,
                                    rhs=kT[c][r0 : r0 + 64, k0 + cc * 512 : min(k0 + (cc + 1) * 512, k1)],
                                    start=True,
                                    stop=True,
                                )
                            pace(pidx, fd)
                            col = (h * 8 + t) * 3 + piece
                            if (pidx * DVE_SH) // 192 != ((pidx + 1) * DVE_SH) // 192:
                                # DVE bit-trick path
                                nc.vector.tensor_scalar(
                                    out=scr_i16[:, 0:fd],
                                    in0=s[:, 0:fd],
                                    scalar1=SC16,
                                    scalar2=SB16,
                                    op0=ALU.mult,
                                    op1=ALU.add,
                                )
                                hf = fd // 2
                                nc.vector.tensor_tensor(
                                    out=tr1[:, 0 : hf // 1],
                                    in0=scr_i16[:, 0:hf].bitcast(bf16),
                                    in1=scr_i16[:, hf:fd].bitcast(bf16),
                                    op=ALU.add,
                                )
                                qf = hf // 2
                                nc.vector.tensor_tensor(
                                    out=tr2[:, 0:qf],
                                    in0=tr1[:, 0:qf],
                                    in1=tr1[:, qf:hf],
                                    op=ALU.add,
                                )
                                nc.vector.tensor_scalar(
                                    out=tr3[:, 0:qf],
                                    in0=tr2[:, 0:qf],
                                    scalar1=1.0,
                                    scalar2=None,
                                    op0=ALU.mult,
                                    op1=ALU.add,
                                    accum_out=lse_acc[:, col : col + 1],
                                )
                            else:
                                nc.scalar.activation(
                                    out=s[:, 0:fd],
                                    in_=s[:, 0:fd],
                                    func=AF.Exp,
                                    scale=SCALE,
                                    accum_out=lse_acc[:, col : col + 1],
                                )
                            pidx += 1
                # flush leftover rides + drains (post-stream)
                while drainq:
                    drainq.pop(0)[1]()
                while state["ride"] < len(rides):
                    _, mm_fn, drain_fn = rides[state["ride"]]
                    if mm_fn is not None:
                        mm_fn()
                    if drain_fn is not None:
                        drain_fn()
                    state["ride"] += 1

                # ---- tail: Ln + rank-1 + projection ------------------------
                # lse: sum the 3 per-piece accumulator cols -> [128, 64]
                la = lse_acc[:].rearrange("q (p three) -> q p three", three=3)
                nc.vector.tensor_add(lse_sum[:], la[:, :, 0], la[:, :, 1])
                nc.vector.tensor_add(lse_sum[:], lse_sum[:], la[:, :, 2])
                nc.scalar.activation(out=lse_ln[:], in_=lse_sum[:], func=AF.Ln)
                lse_tm = const.tile([128, 64], bf16, tag="lse_tm")
                nc.vector.tensor_copy(
                    lse_tm[:],
                    lse_ln[:].rearrange("q (h t) -> q t h", t=QT),
                )
                for t in range(QT):
                    sl = slot[t % 2]
                    ps_bf = sl[0:8, 0:64].bitcast(bf16)
                    nc.tensor.transpose(ps_bf, lse_tm[:, t * 8 : (t + 1) * 8], ident_bf[:])
                    nc.vector.tensor_copy(lnST[:, t * 128 : (t + 1) * 128], ps_bf)
                    yps = sl[:, 512:1024]
                    for c in range(4):
                        nc.tensor.matmul(
                            yps,
                            lhsT=qT[c][:, t * 128 : (t + 1) * 128],
                            rhs=KVW[c][:],
                            start=(c == 0),
                            stop=False,
                        )
                    nc.tensor.matmul(
                        yps,
                        lhsT=lnST[:, t * 128 : (t + 1) * 128],
                        rhs=W8_sb[:],
                        start=False,
                        stop=True,
                    )
                    ysb = dout.tile([128, D], f32, name="ysb", tag="ysb")
                    nc.vector.tensor_add(ysb[:], yps, xb[t][:])
                    nc.sync.dma_start(out=out_d[t * 128 : (t + 1) * 128, :], in_=ysb[:])

    nc.compile()
    return nc


def get_graph():
    if "nc" not in _GRAPH_CACHE:
        _GRAPH_CACHE["nc"] = _build_graph()
    return _GRAPH_CACHE["nc"]


def make_in_maps(x, w_qkv, w_out, b_out):
    import ml_dtypes

    x = np.ascontiguousarray(x, dtype=np.float32)
    w_qkv = np.ascontiguousarray(w_qkv, dtype=np.float32)
    w_out = np.ascontiguousarray(w_out, dtype=np.float32)
    b_out = np.ascontiguousarray(b_out, dtype=np.float32)
    x_bf = x.astype(ml_dtypes.bfloat16)
    w_qkv_bf = w_qkv.astype(ml_dtypes.bfloat16)
    w_out_bf = w_out.astype(ml_dtypes.bfloat16)
    in_maps = []
    for i in range(8):
        b, q = divmod(i, 4)
        in_maps.append(
            {
                # keys are permutation-invariant for lse/kv/G; roll so this
                # core's own query rows sit at rows 0:NQ
                "x_bf": np.ascontiguousarray(np.roll(x_bf[b], -q * NQ, axis=0)),
                "xq": np.ascontiguousarray(x[b, q * NQ : (q + 1) * NQ]),
                "w_qkv_bf": w_qkv_bf,
                "w_out_bf": w_out_bf,
                "b_out": b_out,
            }
        )
    return in_maps


def kernel(x, w_qkv, w_out, b_out):
    from concourse.bass_utils import run_bass_kernel_spmd

    nc = get_graph()
    in_maps = make_in_maps(x, w_qkv, w_out, b_out)
    res = run_bass_kernel_spmd(nc, in_maps, core_ids=list(range(8)))
    out = np.empty((B, N, D), np.float32)
    for i in range(8):
        b, q = divmod(i, 4)
        out[b, q * NQ : (q + 1) * NQ] = res.results[i]["out"]
    return out


# revision 67
# speedup vs baseline: 1.0383x; 1.0037x over previous
"""Trainium2 Bass kernel for log-softmax multi-head attention (8 NeuronCores).

Reference computation (per batch):
    qkv = x @ w_qkv ; q,k,v per head
    dots = scale * q @ k^T ; attn = log_softmax(dots)
    out = attn @ v  -> merge heads -> out @ w_out + b_out + x

Algebraic identities used:
  1) log_softmax is linear in scores minus a row constant:
       attn = scale*dots - lse,  lse_i = ln sum_j exp(scale*dots_ij)
     so  out_head = scale * q @ (k^T v) - lse (x) colsum(v)
  2) k^T v = Wk^T (x^T x) Wv  (Gram matrix G = x^T x shared by all heads)
  3) colsum(v) = colsum(x) @ Wv
  4) the lse rank-1 correction commutes with the output projection
  so the only O(n^2) work is the score matmul + exp/row-sum pass.

Sharding: 8 cores = 2 batches x 4 query-quarters, outputs disjoint.

Schedule (v2): the exp+rowsum pass is split across ScalarE (exact Exp
ACTIVATE, ~0.95ns/elem + 455ns/instr) and DVE (Schraudolph bit-trick:
i16 = trunc(raw*SC16+SB16) is bf16-bitcast ~exp(SCALE*raw); one 1x pass
from PSUM + a 2x bf16 tensor_tensor halving tree + small cache-reduce).
PSUM = 2 stream slots of [128,1536] (3 banks each) + 2 ride banks, so
score tiles are (1536,1536,1024) pieces per 128-row block.  kT/qT/Gram/
GWk/kv ride on the 2 spare banks during the stream (Gram accumulates
32-matmul chains fully in PSUM); rides keep PE continuously busy which
holds it at the 2.4 GHz p-state (idle PE decays to 1.2 GHz).  Tail does
csx/vsum/W8/Ln + rank-1 correction + output projection as before.
"""

import numpy as np

B, N, D = 2, 4096, 512
H, DH = 8, 64
SCALE = DH**-0.5
NQ = N // 4        # own query rows per core
QT = NQ // 128     # 8 own row tiles
NXT = N // 128     # 32 x row tiles

# Schraudolph fast-exp in bf16 (folding the 1/sqrt(DH) score scale):
#   i16 = trunc(raw * SC16 + SB16); bitcast_bf16(i16) ~ exp(SCALE*raw)
# c16=6.9 zeros the mean relative error (max ~4%/elem, ~0.2% on lse)
SC16 = float(np.float32(SCALE * (2.0**7) / np.log(2.0)))
SB16 = float(np.float32(127.0 * 2.0**7 - 6.9))

# stream piece layout per 128-query-row block: key ranges (1024 first so
# the stream can start once transpose-wave 0 of xT has landed)
PIECES = [(0, 1024), (1024, 2560), (2560, 4096)]

_GRAPH_CACHE = {}


def _build_graph():
    import concourse.bass as bass
    import concourse.tile as tile
    from concourse import bacc, mybir
    from concourse.masks import make_identity

    f32 = mybir.dt.float32
    bf16 = mybir.dt.bfloat16
    i16 = mybir.dt.int16
    AF = mybir.ActivationFunctionType
    ALU = mybir.AluOpType

    nc = bacc.Bacc("TRN2", target_bir_lowering=False, debug=False)

    xbf_d = nc.dram_tensor("x_bf", [N, D], bf16, kind="ExternalInput").ap()
    xq_d = nc.dram_tensor("xq", [NQ, D], f32, kind="ExternalInput").ap()
    wqkv_d = nc.dram_tensor("w_qkv_bf", [D, 3 * D], bf16, kind="ExternalInput").ap()
    wout_d = nc.dram_tensor("w_out_bf", [D, D], bf16, kind="ExternalInput").ap()
    bout_d = nc.dram_tensor("b_out", [D], f32, kind="ExternalInput").ap()
    out_d = nc.dram_tensor("out", [NQ, D], f32, kind="ExternalOutput").ap()

    with tile.TileContext(nc) as tc:
        with (
            tc.tile_pool(name="const", bufs=1) as const,
            tc.tile_pool(name="bigsb", bufs=1) as bigsb,
            tc.tile_pool(name="dout", bufs=2) as dout,
        ):
            # ------- constants + DMAs (issue order = priority) --------------
            ident_bf = const.tile([128, 128], bf16, tag="ident_bf")
            make_identity(nc, ident_bf[:])
            wq = []
            for j in range(4):
                w_t = const.tile([128, 3 * D], bf16, tag=f"wq{j}")
                nc.sync.dma_start(out=w_t[:], in_=wqkv_d[j * 128 : (j + 1) * 128, :])
                wq.append(w_t)
            # transposed x (kT/qT source): halves 0:2048 first -> kT0/qT0;
            # merged [128,2048] transpose DMAs (Sync issue cost ~1.3us each)
            xT = [bigsb.tile([128, N], bf16, name=f"xT{j}", tag=f"xT{j}") for j in range(4)]
            for r in range(4):
                for j in range(4):
                    nc.sync.dma_start(
                        out=xT[j][:, r * 1024 : (r + 1) * 1024],
                        in_=xbf_d[r * 1024 : (r + 1) * 1024, j * 128 : (j + 1) * 128],
                        transpose=True,
                    )
            # x row tiles (Gram source; gram rides run late in the stream)
            xrow = []
            for t in range(NXT):
                xr_t = bigsb.tile([128, D], bf16, tag=f"xrow{t}")
                nc.sync.dma_start(out=xr_t[:], in_=xbf_d[t * 128 : (t + 1) * 128, :])
                xrow.append(xr_t)
            wo = []
            for j in range(4):
                w_t = const.tile([128, D], bf16, tag=f"wo{j}")
                nc.sync.dma_start(out=w_t[:], in_=wout_d[j * 128 : (j + 1) * 128, :])
                wo.append(w_t)
            b_bc = const.tile([128, D], f32, tag="b_bc")
            nc.sync.dma_start(
                out=b_bc[:],
                in_=bass.AP(
                    tensor=bout_d.tensor,
                    offset=bout_d.offset,
                    ap=[[0, 128]] + [list(p) for p in bout_d.ap],
                ),
            )
            # residual rows (f32) -> become x + b via DVE adds in tail
            xb = []
            for t in range(QT):
                xb_t = dout.tile([128, D], f32, tag=f"xb{t}", bufs=1)
                nc.sync.dma_start(out=xb_t[:], in_=xq_d[t * 128 : (t + 1) * 128, :])
                xb.append(xb_t)

            # ------- big SBUF operands --------------------------------------
            qT = [bigsb.tile([128, NQ], bf16, name=f"qT{c}", tag=f"qT{c}") for c in range(4)]
            kT = [bigsb.tile([128, N], bf16, name=f"kT{c}", tag=f"kT{c}") for c in range(4)]
            G_sb = [bigsb.tile([128, D], f32, name=f"G{j}", tag=f"G{j}") for j in range(4)]
            G_bf = [bigsb.tile([128, D], bf16, name=f"Gb{j}", tag=f"Gb{j}") for j in range(4)]
            GWk = [bigsb.tile([128, D], bf16, name=f"GWk{j}", tag=f"GWk{j}") for j in range(4)]
            KVW = [bigsb.tile([128, D], bf16, name=f"KVW{c}", tag=f"KVW{c}") for c in range(4)]
            kv_p = const.tile([128, D], bf16, tag="kv_p")
            nc.vector.memset(kv_p[:], 0.0)
            csx4 = [const.tile([128, 4], f32, name=f"csx4_{j}", tag=f"csx4_{j}") for j in range(4)]
            csx_bf = [const.tile([128, 1], bf16, name=f"csxb{j}", tag=f"csxb{j}") for j in range(4)]
            vsT = [const.tile([128, 1], bf16, name=f"vsT{j}", tag=f"vsT{j}") for j in range(4)]
            VSmat = [const.tile([128, 8], bf16, name=f"VSm{j}", tag=f"VSm{j}") for j in range(4)]
            for j in range(4):
                nc.vector.memset(VSmat[j][:], 0.0)
            W8_sb = const.tile([8, D], bf16, tag="W8")
            # lse accumulator: col = (h*8+t)*3 + piece
            lse_acc = const.tile([128, 192], f32, tag="lse_acc")
            lse_sum = const.tile([128, 64], f32, tag="lse_sum")
            lse_ln = const.tile([128, 64], bf16, tag="lse_ln")
            lnST = const.tile([8, NQ], bf16, tag="lnST")
            dummy = const.tile([128, 1], f32, tag="dummy")
            nc.vector.memset(dummy[:], 0.0)
            # bit-trick scratch
            scr_i16 = const.tile([128, 1536], i16, tag="scr_i16")
            tr1 = const.tile([128, 768], bf16, tag="tr1")
            tr2 = const.tile([128, 384], bf16, tag="tr2")
            tr3 = const.tile([128, 384], bf16, tag="tr3")

            # preload the Exp table set before the stream
            nc.scalar.activation(out=dummy[:], in_=dummy[:], func=AF.Exp)

            # =================================================================
            # The single PSUM layout for the whole kernel:
            #   P[:, 0:1536]     slot 0 (3 banks)
            #   P[:, 1536:3072]  slot 1 (3 banks)
            #   P[:, 3072:3584]  ride bank A
            #   P[:, 3584:4096]  ride bank B
            # =================================================================
            with tc.tile_pool(name="allps", bufs=1, space="PSUM") as aps:
                P = aps.tile([128, 4096], f32, tag="P")
                slot = [P[:, 0:1536], P[:, 1536:3072]]
                # one ride bank + one filler bank: filler matmuls are
                # zero-dependency PE work that keeps the tensor engine
                # issue stream gap-free so it holds the 2.4 GHz p-state
                rbank = [P[:, 3072:3584], P[:, 3072:3584]]
                fbank = P[:, 3584:4096]

                # ---- ride bodies, split into MM part + lagged drain part ---
                # (drains issued ~2 pieces after their matmuls so they never
                # head-of-line block the consumer engines)
                def kT_mm(c, ch):
                    for j in range(4):
                        nc.tensor.matmul(
                            rbank[0],
                            lhsT=wq[j][:, 512 + c * 128 : 512 + (c + 1) * 128],
                            rhs=xT[j][:, ch * 512 : (ch + 1) * 512],
                            start=(j == 0),
                            stop=(j == 3),
                        )

                def kT_drain(c, ch, d):
                    if d == 0:
                        nc.vector.tensor_copy(kT[c][:, ch * 512 : (ch + 1) * 512], rbank[0])
                    else:
                        nc.scalar.activation(
                            out=kT[c][:, ch * 512 : (ch + 1) * 512], in_=rbank[0],
                            func=AF.Copy,
                        )

                def qT_mm(c, nn):
                    for j in range(4):
                        nc.tensor.matmul(
                            rbank[0],
                            lhsT=wq[j][:, c * 128 : (c + 1) * 128],
                            rhs=xT[j][:, nn * 512 : (nn + 1) * 512],
                            start=(j == 0),
                            stop=(j == 3),
                        )

                def qT_drain(c, nn, d):
                    if d == 0:
                        nc.vector.tensor_copy(qT[c][:, nn * 512 : (nn + 1) * 512], rbank[0])
                    else:
                        nc.scalar.activation(
                            out=qT[c][:, nn * 512 : (nn + 1) * 512], in_=rbank[0],
                            func=AF.Copy,
                        )

                def gram_mm(jm, t0):
                    for t in range(t0, t0 + 16):
                        nc.tensor.matmul(
                            rbank[0],
                            lhsT=xrow[t][:, jm * 128 : (jm + 1) * 128],
                            rhs=xrow[t][:],
                            start=(t == t0),
                            stop=(t == t0 + 15),
                        )

                def gram_drain(jm, t0):
                    if t0 == 0:
                        nc.vector.tensor_copy(G_sb[jm][:], rbank[0])
                    else:
                        nc.vector.tensor_add(G_sb[jm][:], G_sb[jm][:], rbank[0])
                        nc.vector.tensor_copy(G_bf[jm][:], G_sb[jm][:])

                def gwk_mm(jm):
                    for j in range(4):
                        nc.tensor.matmul(
                            rbank[0],
                            lhsT=G_bf[j][:, jm * 128 : (jm + 1) * 128],
                            rhs=wq[j][:, 512:1024],
                            start=(j == 0),
                            stop=(j == 3),
                        )

                def gwk_drain(jm, d):
                    if d == 0:
                        nc.vector.tensor_copy(GWk[jm][:], rbank[0])
                    else:
                        nc.scalar.activation(out=GWk[jm][:], in_=rbank[0], func=AF.Copy)

                def kvt_mm(h):
                    for j in range(4):
                        nc.tensor.matmul(
                            rbank[0][0:64, 0:64],
                            lhsT=wq[j][:, 1024 + h * 64 : 1024 + (h + 1) * 64],
                            rhs=GWk[j][:, h * 64 : (h + 1) * 64],
                            start=(j == 0),
                            stop=(j == 3),
                        )

                def kvt_drain(h):
                    r0 = (h % 2) * 64
                    nc.vector.tensor_scalar_mul(
                        kv_p[r0 : r0 + 64, h * 64 : (h + 1) * 64],
                        rbank[0][0:64, 0:64], SCALE,
                    )

                # ---- head: kT0 ch0-1 + qT0 (xT wave 0 + wq only) -----------
                kT_mm(0, 0)
                kT_drain(0, 0, 0)
                kT_mm(0, 1)
                kT_drain(0, 1, 1)
                qT_mm(0, 0)
                qT_drain(0, 0, 0)
                qT_mm(0, 1)
                qT_drain(0, 1, 1)

                # ---- ride schedule: (target piece, mm_fn, drain_fn) --------
                # single ride bank; ordered by deadline (kT[c] first needed
                # at piece 48c; gram/gwk/kv only by the tail)
                rides = []
                for i, ch in enumerate(range(2, 8)):         # kT0 tail
                    rides.append((2 * (i + 1),
                                  lambda ch=ch: kT_mm(0, ch),
                                  lambda ch=ch, d=ch % 2: kT_drain(0, ch, d)))
                for c in range(1, 4):                        # kT1-3 + qT1-3
                    base = 36 * c - 20
                    for ch in range(8):
                        rides.append((base + 3 * ch,
                                      lambda c=c, ch=ch: kT_mm(c, ch),
                                      lambda c=c, ch=ch, d=(ch + c) % 2: kT_drain(c, ch, d)))
                    rides.append((base + 24, lambda c=c: qT_mm(c, 0),
                                  lambda c=c: qT_drain(c, 0, 0)))
                    rides.append((base + 27, lambda c=c: qT_mm(c, 1),
                                  lambda c=c: qT_drain(c, 1, 1)))
                for jm in range(4):                          # Gram halves
                    for s16 in range(2):
                        rides.append((112 + 6 * jm + 3 * s16,
                                      lambda jm=jm, t0=16 * s16: gram_mm(jm, t0),
                                      lambda jm=jm, t0=16 * s16: gram_drain(jm, t0)))
                for jm in range(4):                          # G @ Wk
                    rides.append((140 + 3 * jm, lambda jm=jm: gwk_mm(jm),
                                  lambda jm=jm, d=jm % 2: gwk_drain(jm, d)))
                for h in range(H):                           # kv per head
                    rides.append((152 + 2 * h, lambda h=h: kvt_mm(h),
                                  lambda h=h: kvt_drain(h)))

                # late-stream DVE/PE rides: csx, xb, vsum, W8, KVW (the only
                # true tail is the Ln -> rank-1 -> projection chain)
                def csx_piece(j, p):
                    nc.vector.tensor_reduce(
                        csx4[j][:, p : p + 1],
                        xT[j][:, p * 1024 : (p + 1) * 1024],
                        axis=mybir.AxisListType.X,
                        op=ALU.add,
                    )

                def csx_fin(j):
                    nc.vector.tensor_reduce(
                        csx4[j][:, 0:1], csx4[j][:],
                        axis=mybir.AxisListType.X, op=ALU.add,
                    )
                    nc.vector.tensor_copy(csx_bf[j][:], csx4[j][:, 0:1])

                def xb_add(t):
                    nc.vector.tensor_add(xb[t][:], xb[t][:], b_bc[:])

                def vsum_mm(jm):
                    for j in range(4):
                        nc.tensor.matmul(
                            rbank[0][:, 0:1],
                            lhsT=wq[j][:, 1024 + jm * 128 : 1024 + (jm + 1) * 128],
                            rhs=csx_bf[j][:],
                            start=(j == 0),
                            stop=(j == 3),
                        )

                def vsum_drain(jm):
                    nc.vector.tensor_scalar_mul(vsT[jm][:], rbank[0][:, 0:1], -1.0)
                    nc.vector.tensor_copy(
                        VSmat[jm][0:64, 2 * jm : 2 * jm + 1], vsT[jm][0:64, :]
                    )
                    nc.vector.tensor_copy(
                        VSmat[jm][64:128, 2 * jm + 1 : 2 * jm + 2], vsT[jm][64:128, :]
                    )

                def w8_mm():
                    for j in range(4):
                        nc.tensor.matmul(
                            rbank[0][0:8, :],
                            lhsT=VSmat[j][:],
                            rhs=wo[j][:],
                            start=(j == 0),
                            stop=(j == 3),
                        )

                def w8_drain():
                    nc.vector.tensor_copy(W8_sb[:], rbank[0][0:8, :])

                def kvw_mm(c):
                    for hp in range(2):
                        h, r0h = 2 * c + hp, hp * 64
                        nc.tensor.matmul(
                            rbank[0][r0h : r0h + 64, :],
                            lhsT=kv_p[:, h * 64 : (h + 1) * 64],
                            rhs=wo[c][:],
                            start=True,
                            stop=True,
                        )

                def kvw_drain(c):
                    nc.vector.tensor_copy(KVW[c][:], rbank[0])

                for i in range(16):
                    rides.append((118 + 2 * i, None,
                                  lambda j=i // 4, p=i % 4: csx_piece(j, p)))
                for j in range(4):
                    rides.append((151 + j, None, lambda j=j: csx_fin(j)))
                for t in range(QT):
                    rides.append((139 + 2 * t, None, lambda t=t: xb_add(t)))
                for jm in range(4):
                    rides.append((155 + 3 * jm, lambda jm=jm: vsum_mm(jm),
                                  lambda jm=jm: vsum_drain(jm)))
                rides.append((183, w8_mm, w8_drain))
                for c in range(4):
                    rides.append((168 + 3 * c, lambda c=c: kvw_mm(c),
                                  lambda c=c: kvw_drain(c)))
                rides.sort(key=lambda r: r[0])

                # ---- the exp stream: (h, piece, t); rides + filler paced --
                # DVE takes DVE_SH of 192 pieces; filler matmuls keep the PE
                # issue stream from idling (p-state)
                DVE_SH = 62
                LAG = 0
                state = {"ride": 0}
                drainq = []

                def filler_mm():
                    nc.tensor.matmul(
                        fbank, lhsT=wq[0][:, 0:128], rhs=wq[0][:, 0:512],
                        start=True, stop=True,
                    )

                def pace(pidx, dots_cols):
                    # drains from rides issued >= LAG pieces ago: their
                    # matmuls are long retired, so the consumer-engine copy
                    # dispatches without stalling the ops behind it
                    while drainq and pidx >= drainq[0][0]:
                        drainq.pop(0)[1]()
                    nmm = 0
                    while state["ride"] < len(rides) and pidx >= rides[state["ride"]][0]:
                        tgt, mm_fn, drain_fn = rides[state["ride"]]
                        if mm_fn is not None:
                            # program order must keep the pending drain
                            # before the next bank write
                            if drainq:
                                break
                            mm_fn()
                            nmm = 1
                            if drain_fn is not None:
                                drainq.append((pidx + LAG, drain_fn))
                        elif drain_fn is not None:
                            drain_fn()
                        state["ride"] += 1
                        if nmm:
                            break  # one bank ride per piece max
                    filler_mm()
                    if not nmm:
                        filler_mm()

                pidx = 0
                # group order defers each head-pair's high-key piece 2 until
                # after both heads' pieces 0-1, giving the wave-2/3 transpose
                # DMAs and kT ch5-7 rides ~15 extra pieces of slack (the
                # early stream otherwise stalls on them)
                order = []
                for cc_ in range(4):
                    h0, h1 = 2 * cc_, 2 * cc_ + 1
                    order += [(h0, 0), (h0, 1), (h1, 0), (h1, 1),
                              (h0, 2), (h1, 2)]
                for h, piece in order:
                    c, r0 = h // 2, (h % 2) * 64
                    k0, k1 = PIECES[piece]
                    if True:
                        for t in range(QT):
                            s = slot[pidx % 2]
                            fd = k1 - k0
                            lhsT = qT[c][r0 : r0 + 64, t * 128 : (t + 1) * 128]
                            for cc in range((fd + 511) // 512):
                                nc.tensor.matmul(
                                    s[:, cc * 512 : min((cc + 1) * 512, fd)],
                                    lhsT=lhsT,
                                    rhs=kT[c][r0 : r0 + 64, k0 + cc * 512 : min(k0 + (cc + 1) * 512, k1)],
                                    start=True,
                                    stop=True,
                                )
                            pace(pidx, fd)
                            col = (h * 8 + t) * 3 + piece
                            if (pidx * DVE_SH) // 192 != ((pidx + 1) * DVE_SH) // 192:
                                # DVE bit-trick path
                                nc.vector.tensor_scalar(
                                    out=scr_i16[:, 0:fd],
                                    in0=s[:, 0:fd],
                                    scalar1=SC16,
                                    scalar2=SB16,
                                    op0=ALU.mult,
                                    op1=ALU.add,
                                )
                                hf = fd // 2
                                nc.vector.tensor_tensor(
                                    out=tr1[:, 0 : hf // 1],
                                    in0=scr_i16[:, 0:hf].bitcast(bf16),
                                    in1=scr_i16[:, hf:fd].bitcast(bf16),
                                    op=ALU.add,
                                )
                                qf = hf // 2
                                nc.vector.tensor_tensor(
                                    out=tr2[:, 0:qf],
                                    in0=tr1[:, 0:qf],
                                    in1=tr1[:, qf:hf],
                                    op=ALU.add,
                                )
                                nc.vector.tensor_scalar(
                                    out=tr3[:, 0:qf],
                                    in0=tr2[:, 0:qf],
                                    scalar1=1.0,
                                    scalar2=None,
                                    op0=ALU.mult,
                                    op1=ALU.add,
                                    accum_out=lse_acc[:, col : col + 1],
                                )
                            else:
                                nc.scalar.activation(
                                    out=s[:, 0:fd],
                                    in_=s[:, 0:fd],
                                    func=AF.Exp,
                                    scale=SCALE,
                                    accum_out=lse_acc[:, col : col + 1],
                                )
                            pidx += 1
                # flush leftover rides + drains (post-stream)
                while drainq:
                    drainq.pop(0)[1]()
                while state["ride"] < len(rides):
                    _, mm_fn, drain_fn = rides[state["ride"]]
                    if mm_fn is not None:
                        mm_fn()
                    if drain_fn is not None:
                        drain_fn()
                    state["ride"] += 1

                # ---- tail: Ln + rank-1 + projection ------------------------
                # lse: sum the 3 per-piece accumulator cols -> [128, 64]
                la = lse_acc[:].rearrange("q (p three) -> q p three", three=3)
                nc.vector.tensor_add(lse_sum[:], la[:, :, 0], la[:, :, 1])
                nc.vector.tensor_add(lse_sum[:], lse_sum[:], la[:, :, 2])
                nc.scalar.activation(out=lse_ln[:], in_=lse_sum[:], func=AF.Ln)
                lse_tm = const.tile([128, 64], bf16, tag="lse_tm")
                nc.vector.tensor_copy(
                    lse_tm[:],
                    lse_ln[:].rearrange("q (h t) -> q t h", t=QT),
                )
                for t in range(QT):
                    sl = slot[t % 2]
                    ps_bf = sl[0:8, 0:64].bitcast(bf16)
                    nc.tensor.transpose(ps_bf, lse_tm[:, t * 8 : (t + 1) * 8], ident_bf[:])
                    nc.vector.tensor_copy(lnST[:, t * 128 : (t + 1) * 128], ps_bf)
                    yps = sl[:, 512:1024]
                    for c in range(4):
                        nc.tensor.matmul(
                            yps,
                            lhsT=qT[c][:, t * 128 : (t + 1) * 128],
                            rhs=KVW[c][:],
                            start=(c == 0),
                            stop=False,
                        )
                    nc.tensor.matmul(
                        yps,
                        lhsT=lnST[:, t * 128 : (t + 1) * 128],
                        rhs=W8_sb[:],
                        start=False,
                        stop=True,
                    )
                    ysb = dout.tile([128, D], f32, name="ysb", tag="ysb")
                    nc.vector.tensor_add(ysb[:], yps, xb[t][:])
                    nc.sync.dma_start(out=out_d[t * 128 : (t + 1) * 128, :], in_=ysb[:])

    nc.compile()
    return nc


def get_graph():
    if "nc" not in _GRAPH_CACHE:
        _GRAPH_CACHE["nc"] = _build_graph()
    return _GRAPH_CACHE["nc"]


def make_in_maps(x, w_qkv, w_out, b_out):
    import ml_dtypes

    x = np.ascontiguousarray(x, dtype=np.float32)
    w_qkv = np.ascontiguousarray(w_qkv, dtype=np.float32)
    w_out = np.ascontiguousarray(w_out, dtype=np.float32)
    b_out = np.ascontiguousarray(b_out, dtype=np.float32)
    x_bf = x.astype(ml_dtypes.bfloat16)
    w_qkv_bf = w_qkv.astype(ml_dtypes.bfloat16)
    w_out_bf = w_out.astype(ml_dtypes.bfloat16)
    in_maps = []
    for i in range(8):
        b, q = divmod(i, 4)
        in_maps.append(
            {
                # keys are permutation-invariant for lse/kv/G; roll so this
                # core's own query rows sit at rows 0:NQ
                "x_bf": np.ascontiguousarray(np.roll(x_bf[b], -q * NQ, axis=0)),
                "xq": np.ascontiguousarray(x[b, q * NQ : (q + 1) * NQ]),
                "w_qkv_bf": w_qkv_bf,
                "w_out_bf": w_out_bf,
                "b_out": b_out,
            }
        )
    return in_maps


def kernel(x, w_qkv, w_out, b_out):
    from concourse.bass_utils import run_bass_kernel_spmd

    nc = get_graph()
    in_maps = make_in_maps(x, w_qkv, w_out, b_out)
    res = run_bass_kernel_spmd(nc, in_maps, core_ids=list(range(8)))
    out = np.empty((B, N, D), np.float32)
    for i in range(8):
        b, q = divmod(i, 4)
        out[b, q * NQ : (q + 1) * NQ] = res.results[i]["out"]
    return out


# revision 68
# speedup vs baseline: 1.0384x; 1.0001x over previous
"""Trainium2 Bass kernel for log-softmax multi-head attention (8 NeuronCores).

Reference computation (per batch):
    qkv = x @ w_qkv ; q,k,v per head
    dots = scale * q @ k^T ; attn = log_softmax(dots)
    out = attn @ v  -> merge heads -> out @ w_out + b_out + x

Algebraic identities used:
  1) log_softmax is linear in scores minus a row constant:
       attn = scale*dots - lse,  lse_i = ln sum_j exp(scale*dots_ij)
     so  out_head = scale * q @ (k^T v) - lse (x) colsum(v)
  2) k^T v = Wk^T (x^T x) Wv  (Gram matrix G = x^T x shared by all heads)
  3) colsum(v) = colsum(x) @ Wv
  4) the lse rank-1 correction commutes with the output projection
  so the only O(n^2) work is the score matmul + exp/row-sum pass.

Sharding: 8 cores = 2 batches x 4 query-quarters, outputs disjoint.

Schedule (v2): the exp+rowsum pass is split across ScalarE (exact Exp
ACTIVATE, ~0.95ns/elem + 455ns/instr) and DVE (Schraudolph bit-trick:
i16 = trunc(raw*SC16+SB16) is bf16-bitcast ~exp(SCALE*raw); one 1x pass
from PSUM + a 2x bf16 tensor_tensor halving tree + small cache-reduce).
PSUM = 2 stream slots of [128,1536] (3 banks each) + 2 ride banks, so
score tiles are (1536,1536,1024) pieces per 128-row block.  kT/qT/Gram/
GWk/kv ride on the 2 spare banks during the stream (Gram accumulates
32-matmul chains fully in PSUM); rides keep PE continuously busy which
holds it at the 2.4 GHz p-state (idle PE decays to 1.2 GHz).  Tail does
csx/vsum/W8/Ln + rank-1 correction + output projection as before.
"""

import numpy as np

B, N, D = 2, 4096, 512
H, DH = 8, 64
SCALE = DH**-0.5
NQ = N // 4        # own query rows per core
QT = NQ // 128     # 8 own row tiles
NXT = N // 128     # 32 x row tiles

# Schraudolph fast-exp in bf16 (folding the 1/sqrt(DH) score scale):
#   i16 = trunc(raw * SC16 + SB16); bitcast_bf16(i16) ~ exp(SCALE*raw)
# c16=6.9 zeros the mean relative error (max ~4%/elem, ~0.2% on lse)
SC16 = float(np.float32(SCALE * (2.0**7) / np.log(2.0)))
SB16 = float(np.float32(127.0 * 2.0**7 - 6.9))

# stream piece layout per 128-query-row block: key ranges (1024 first so
# the stream can start once transpose-wave 0 of xT has landed)
PIECES = [(0, 1024), (1024, 2560), (2560, 4096)]

_GRAPH_CACHE = {}


def _build_graph():
    import concourse.bass as bass
    import concourse.tile as tile
    from concourse import bacc, mybir
    from concourse.masks import make_identity

    f32 = mybir.dt.float32
    bf16 = mybir.dt.bfloat16
    i16 = mybir.dt.int16
    AF = mybir.ActivationFunctionType
    ALU = mybir.AluOpType

    nc = bacc.Bacc("TRN2", target_bir_lowering=False, debug=False)

    xbf_d = nc.dram_tensor("x_bf", [N, D], bf16, kind="ExternalInput").ap()
    xq_d = nc.dram_tensor("xq", [NQ, D], f32, kind="ExternalInput").ap()
    wqkv_d = nc.dram_tensor("w_qkv_bf", [D, 3 * D], bf16, kind="ExternalInput").ap()
    wout_d = nc.dram_tensor("w_out_bf", [D, D], bf16, kind="ExternalInput").ap()
    bout_d = nc.dram_tensor("b_out", [D], f32, kind="ExternalInput").ap()
    out_d = nc.dram_tensor("out", [NQ, D], f32, kind="ExternalOutput").ap()

    with tile.TileContext(nc) as tc:
        with (
            tc.tile_pool(name="const", bufs=1) as const,
            tc.tile_pool(name="bigsb", bufs=1) as bigsb,
            tc.tile_pool(name="dout", bufs=2) as dout,
        ):
            # ------- constants + DMAs (issue order = priority) --------------
            ident_bf = const.tile([128, 128], bf16, tag="ident_bf")
            make_identity(nc, ident_bf[:])
            wq = []
            for j in range(4):
                w_t = const.tile([128, 3 * D], bf16, tag=f"wq{j}")
                nc.sync.dma_start(out=w_t[:], in_=wqkv_d[j * 128 : (j + 1) * 128, :])
                wq.append(w_t)
            # transposed x (kT/qT source): halves 0:2048 first -> kT0/qT0;
            # merged [128,2048] transpose DMAs (Sync issue cost ~1.3us each)
            xT = [bigsb.tile([128, N], bf16, name=f"xT{j}", tag=f"xT{j}") for j in range(4)]
            for r in range(4):
                for j in range(4):
                    nc.sync.dma_start(
                        out=xT[j][:, r * 1024 : (r + 1) * 1024],
                        in_=xbf_d[r * 1024 : (r + 1) * 1024, j * 128 : (j + 1) * 128],
                        transpose=True,
                    )
            # x row tiles (Gram source; gram rides run late in the stream)
            xrow = []
            for t in range(NXT):
                xr_t = bigsb.tile([128, D], bf16, tag=f"xrow{t}")
                nc.sync.dma_start(out=xr_t[:], in_=xbf_d[t * 128 : (t + 1) * 128, :])
                xrow.append(xr_t)
            wo = []
            for j in range(4):
                w_t = const.tile([128, D], bf16, tag=f"wo{j}")
                nc.sync.dma_start(out=w_t[:], in_=wout_d[j * 128 : (j + 1) * 128, :])
                wo.append(w_t)
            b_bc = const.tile([128, D], f32, tag="b_bc")
            nc.sync.dma_start(
                out=b_bc[:],
                in_=bass.AP(
                    tensor=bout_d.tensor,
                    offset=bout_d.offset,
                    ap=[[0, 128]] + [list(p) for p in bout_d.ap],
                ),
            )
            # residual rows (f32) -> become x + b via DVE adds in tail
            xb = []
            for t in range(QT):
                xb_t = dout.tile([128, D], f32, tag=f"xb{t}", bufs=1)
                nc.sync.dma_start(out=xb_t[:], in_=xq_d[t * 128 : (t + 1) * 128, :])
                xb.append(xb_t)

            # ------- big SBUF operands --------------------------------------
            qT = [bigsb.tile([128, NQ], bf16, name=f"qT{c}", tag=f"qT{c}") for c in range(4)]
            kT = [bigsb.tile([128, N], bf16, name=f"kT{c}", tag=f"kT{c}") for c in range(4)]
            G_sb = [bigsb.tile([128, D], f32, name=f"G{j}", tag=f"G{j}") for j in range(4)]
            G_bf = [bigsb.tile([128, D], bf16, name=f"Gb{j}", tag=f"Gb{j}") for j in range(4)]
            GWk = [bigsb.tile([128, D], bf16, name=f"GWk{j}", tag=f"GWk{j}") for j in range(4)]
            KVW = [bigsb.tile([128, D], bf16, name=f"KVW{c}", tag=f"KVW{c}") for c in range(4)]
            kv_p = const.tile([128, D], bf16, tag="kv_p")
            nc.vector.memset(kv_p[:], 0.0)
            csx4 = [const.tile([128, 4], f32, name=f"csx4_{j}", tag=f"csx4_{j}") for j in range(4)]
            csx_bf = [const.tile([128, 1], bf16, name=f"csxb{j}", tag=f"csxb{j}") for j in range(4)]
            vsT = [const.tile([128, 1], bf16, name=f"vsT{j}", tag=f"vsT{j}") for j in range(4)]
            VSmat = [const.tile([128, 8], bf16, name=f"VSm{j}", tag=f"VSm{j}") for j in range(4)]
            for j in range(4):
                nc.vector.memset(VSmat[j][:], 0.0)
            W8_sb = const.tile([8, D], bf16, tag="W8")
            # lse accumulator: col = (h*8+t)*3 + piece
            lse_acc = const.tile([128, 192], f32, tag="lse_acc")
            lse_sum = const.tile([128, 64], f32, tag="lse_sum")
            lse_ln = const.tile([128, 64], bf16, tag="lse_ln")
            lnST = const.tile([8, NQ], bf16, tag="lnST")
            dummy = const.tile([128, 1], f32, tag="dummy")
            nc.vector.memset(dummy[:], 0.0)
            # bit-trick scratch
            scr_i16 = const.tile([128, 1536], i16, tag="scr_i16")
            tr1 = const.tile([128, 768], bf16, tag="tr1")
            tr2 = const.tile([128, 384], bf16, tag="tr2")
            tr3 = const.tile([128, 384], bf16, tag="tr3")

            # preload the Exp table set before the stream
            nc.scalar.activation(out=dummy[:], in_=dummy[:], func=AF.Exp)

            # =================================================================
            # The single PSUM layout for the whole kernel:
            #   P[:, 0:1536]     slot 0 (3 banks)
            #   P[:, 1536:3072]  slot 1 (3 banks)
            #   P[:, 3072:3584]  ride bank A
            #   P[:, 3584:4096]  ride bank B
            # =================================================================
            with tc.tile_pool(name="allps", bufs=1, space="PSUM") as aps:
                P = aps.tile([128, 4096], f32, tag="P")
                slot = [P[:, 0:1536], P[:, 1536:3072]]
                # one ride bank + one filler bank: filler matmuls are
                # zero-dependency PE work that keeps the tensor engine
                # issue stream gap-free so it holds the 2.4 GHz p-state
                rbank = [P[:, 3072:3584], P[:, 3072:3584]]
                fbank = P[:, 3584:4096]

                # ---- ride bodies, split into MM part + lagged drain part ---
                # (drains issued ~2 pieces after their matmuls so they never
                # head-of-line block the consumer engines)
                def kT_mm(c, ch):
                    for j in range(4):
                        nc.tensor.matmul(
                            rbank[0],
                            lhsT=wq[j][:, 512 + c * 128 : 512 + (c + 1) * 128],
                            rhs=xT[j][:, ch * 512 : (ch + 1) * 512],
                            start=(j == 0),
                            stop=(j == 3),
                        )

                def kT_drain(c, ch, d):
                    if d == 0:
                        nc.vector.tensor_copy(kT[c][:, ch * 512 : (ch + 1) * 512], rbank[0])
                    else:
                        nc.scalar.activation(
                            out=kT[c][:, ch * 512 : (ch + 1) * 512], in_=rbank[0],
                            func=AF.Copy,
                        )

                def qT_mm(c, nn):
                    for j in range(4):
                        nc.tensor.matmul(
                            rbank[0],
                            lhsT=wq[j][:, c * 128 : (c + 1) * 128],
                            rhs=xT[j][:, nn * 512 : (nn + 1) * 512],
                            start=(j == 0),
                            stop=(j == 3),
                        )

                def qT_drain(c, nn, d):
                    if d == 0:
                        nc.vector.tensor_copy(qT[c][:, nn * 512 : (nn + 1) * 512], rbank[0])
                    else:
                        nc.scalar.activation(
                            out=qT[c][:, nn * 512 : (nn + 1) * 512], in_=rbank[0],
                            func=AF.Copy,
                        )

                def gram_mm(jm, t0):
                    for t in range(t0, t0 + 16):
                        nc.tensor.matmul(
                            rbank[0],
                            lhsT=xrow[t][:, jm * 128 : (jm + 1) * 128],
                            rhs=xrow[t][:],
                            start=(t == t0),
                            stop=(t == t0 + 15),
                        )

                def gram_drain(jm, t0):
                    if t0 == 0:
                        nc.vector.tensor_copy(G_sb[jm][:], rbank[0])
                    else:
                        nc.vector.tensor_add(G_sb[jm][:], G_sb[jm][:], rbank[0])
                        nc.vector.tensor_copy(G_bf[jm][:], G_sb[jm][:])

                def gwk_mm(jm):
                    for j in range(4):
                        nc.tensor.matmul(
                            rbank[0],
                            lhsT=G_bf[j][:, jm * 128 : (jm + 1) * 128],
                            rhs=wq[j][:, 512:1024],
                            start=(j == 0),
                            stop=(j == 3),
                        )

                def gwk_drain(jm, d):
                    if d == 0:
                        nc.vector.tensor_copy(GWk[jm][:], rbank[0])
                    else:
                        nc.scalar.activation(out=GWk[jm][:], in_=rbank[0], func=AF.Copy)

                def kvt_mm(h):
                    for j in range(4):
                        nc.tensor.matmul(
                            rbank[0][0:64, 0:64],
                            lhsT=wq[j][:, 1024 + h * 64 : 1024 + (h + 1) * 64],
                            rhs=GWk[j][:, h * 64 : (h + 1) * 64],
                            start=(j == 0),
                            stop=(j == 3),
                        )

                def kvt_drain(h):
                    r0 = (h % 2) * 64
                    nc.vector.tensor_scalar_mul(
                        kv_p[r0 : r0 + 64, h * 64 : (h + 1) * 64],
                        rbank[0][0:64, 0:64], SCALE,
                    )

                # ---- head: kT0 ch0-1 + qT0 (xT wave 0 + wq only) -----------
                kT_mm(0, 0)
                kT_drain(0, 0, 0)
                kT_mm(0, 1)
                kT_drain(0, 1, 1)
                qT_mm(0, 0)
                qT_drain(0, 0, 0)
                qT_mm(0, 1)
                qT_drain(0, 1, 1)

                # ---- ride schedule: (target piece, mm_fn, drain_fn) --------
                # single ride bank; ordered by deadline (kT[c] first needed
                # at piece 48c; gram/gwk/kv only by the tail)
                rides = []
                for i, ch in enumerate(range(2, 8)):         # kT0 tail
                    rides.append((2 * (i + 1),
                                  lambda ch=ch: kT_mm(0, ch),
                                  lambda ch=ch, d=ch % 2: kT_drain(0, ch, d)))
                for c in range(1, 4):                        # kT1-3 + qT1-3
                    # order by first use at the h=2c boundary: ch0, ch1 and
                    # qT half-0 (dots lhsT) are needed at piece 48c; later
                    # chunks/half-1 have slack from the deferred group order
                    base = 36 * c - 20
                    seq = [("k", 0), ("k", 1), ("q", 0), ("k", 2), ("k", 3),
                           ("k", 4), ("q", 1), ("k", 5), ("k", 6), ("k", 7)]
                    for i, (kind, idx) in enumerate(seq):
                        tgt = base + 3 * i
                        if kind == "k":
                            rides.append((tgt,
                                          lambda c=c, ch=idx: kT_mm(c, ch),
                                          lambda c=c, ch=idx, d=(idx + c) % 2: kT_drain(c, ch, d)))
                        else:
                            rides.append((tgt, lambda c=c, nn=idx: qT_mm(c, nn),
                                          lambda c=c, nn=idx: qT_drain(c, nn, nn)))
                for jm in range(4):                          # Gram halves
                    for s16 in range(2):
                        rides.append((112 + 6 * jm + 3 * s16,
                                      lambda jm=jm, t0=16 * s16: gram_mm(jm, t0),
                                      lambda jm=jm, t0=16 * s16: gram_drain(jm, t0)))
                for jm in range(4):                          # G @ Wk
                    rides.append((140 + 3 * jm, lambda jm=jm: gwk_mm(jm),
                                  lambda jm=jm, d=jm % 2: gwk_drain(jm, d)))
                for h in range(H):                           # kv per head
                    rides.append((152 + 2 * h, lambda h=h: kvt_mm(h),
                                  lambda h=h: kvt_drain(h)))

                # late-stream DVE/PE rides: csx, xb, vsum, W8, KVW (the only
                # true tail is the Ln -> rank-1 -> projection chain)
                def csx_piece(j, p):
                    nc.vector.tensor_reduce(
                        csx4[j][:, p : p + 1],
                        xT[j][:, p * 1024 : (p + 1) * 1024],
                        axis=mybir.AxisListType.X,
                        op=ALU.add,
                    )

                def csx_fin(j):
                    nc.vector.tensor_reduce(
                        csx4[j][:, 0:1], csx4[j][:],
                        axis=mybir.AxisListType.X, op=ALU.add,
                    )
                    nc.vector.tensor_copy(csx_bf[j][:], csx4[j][:, 0:1])

                def xb_add(t):
                    nc.vector.tensor_add(xb[t][:], xb[t][:], b_bc[:])

                def vsum_mm(jm):
                    for j in range(4):
                        nc.tensor.matmul(
                            rbank[0][:, 0:1],
                            lhsT=wq[j][:, 1024 + jm * 128 : 1024 + (jm + 1) * 128],
                            rhs=csx_bf[j][:],
                            start=(j == 0),
                            stop=(j == 3),
                        )

                def vsum_drain(jm):
                    nc.vector.tensor_scalar_mul(vsT[jm][:], rbank[0][:, 0:1], -1.0)
                    nc.vector.tensor_copy(
                        VSmat[jm][0:64, 2 * jm : 2 * jm + 1], vsT[jm][0:64, :]
                    )
                    nc.vector.tensor_copy(
                        VSmat[jm][64:128, 2 * jm + 1 : 2 * jm + 2], vsT[jm][64:128, :]
                    )

                def w8_mm():
                    for j in range(4):
                        nc.tensor.matmul(
                            rbank[0][0:8, :],
                            lhsT=VSmat[j][:],
                            rhs=wo[j][:],
                            start=(j == 0),
                            stop=(j == 3),
                        )

                def w8_drain():
                    nc.vector.tensor_copy(W8_sb[:], rbank[0][0:8, :])

                def kvw_mm(c):
                    for hp in range(2):
                        h, r0h = 2 * c + hp, hp * 64
                        nc.tensor.matmul(
                            rbank[0][r0h : r0h + 64, :],
                            lhsT=kv_p[:, h * 64 : (h + 1) * 64],
                            rhs=wo[c][:],
                            start=True,
                            stop=True,
                        )

                def kvw_drain(c):
                    nc.vector.tensor_copy(KVW[c][:], rbank[0])

                for i in range(16):
                    rides.append((118 + 2 * i, None,
                                  lambda j=i // 4, p=i % 4: csx_piece(j, p)))
                for j in range(4):
                    rides.append((151 + j, None, lambda j=j: csx_fin(j)))
                for t in range(QT):
                    rides.append((139 + 2 * t, None, lambda t=t: xb_add(t)))
                for jm in range(4):
                    rides.append((155 + 3 * jm, lambda jm=jm: vsum_mm(jm),
                                  lambda jm=jm: vsum_drain(jm)))
                rides.append((183, w8_mm, w8_drain))
                for c in range(4):
                    rides.append((168 + 3 * c, lambda c=c: kvw_mm(c),
                                  lambda c=c: kvw_drain(c)))
                rides.sort(key=lambda r: r[0])

                # ---- the exp stream: (h, piece, t); rides + filler paced --
                # DVE takes DVE_SH of 192 pieces; filler matmuls keep the PE
                # issue stream from idling (p-state)
                DVE_SH = 62
                LAG = 0
                state = {"ride": 0}
                drainq = []

                def filler_mm():
                    nc.tensor.matmul(
                        fbank, lhsT=wq[0][:, 0:128], rhs=wq[0][:, 0:512],
                        start=True, stop=True,
                    )

                def pace(pidx, dots_cols):
                    # drains from rides issued >= LAG pieces ago: their
                    # matmuls are long retired, so the consumer-engine copy
                    # dispatches without stalling the ops behind it
                    while drainq and pidx >= drainq[0][0]:
                        drainq.pop(0)[1]()
                    nmm = 0
                    while state["ride"] < len(rides) and pidx >= rides[state["ride"]][0]:
                        tgt, mm_fn, drain_fn = rides[state["ride"]]
                        if mm_fn is not None:
                            # program order must keep the pending drain
                            # before the next bank write
                            if drainq:
                                break
                            mm_fn()
                            nmm = 1
                            if drain_fn is not None:
                                drainq.append((pidx + LAG, drain_fn))
                        elif drain_fn is not None:
                            drain_fn()
                        state["ride"] += 1
                        if nmm:
                            break  # one bank ride per piece max
                    filler_mm()
                    if not nmm:
                        filler_mm()

                pidx = 0
                # group order defers each head-pair's high-key piece 2 until
                # after both heads' pieces 0-1, giving the wave-2/3 transpose
                # DMAs and kT ch5-7 rides ~15 extra pieces of slack (the
                # early stream otherwise stalls on them)
                order = []
                for cc_ in range(4):
                    h0, h1 = 2 * cc_, 2 * cc_ + 1
                    order += [(h0, 0), (h0, 1), (h1, 0), (h1, 1),
                              (h0, 2), (h1, 2)]
                for h, piece in order:
                    c, r0 = h // 2, (h % 2) * 64
                    k0, k1 = PIECES[piece]
                    if True:
                        for t in range(QT):
                            s = slot[pidx % 2]
                            fd = k1 - k0
                            lhsT = qT[c][r0 : r0 + 64, t * 128 : (t + 1) * 128]
                            for cc in range((fd + 511) // 512):
                                nc.tensor.matmul(
                                    s[:, cc * 512 : min((cc + 1) * 512, fd)],
                                    lhsT=lhsT,
                                    rhs=kT[c][r0 : r0 + 64, k0 + cc * 512 : min(k0 + (cc + 1) * 512, k1)],
                                    start=True,
                                    stop=True,
                                )
                            pace(pidx, fd)
                            col = (h * 8 + t) * 3 + piece
                            if (pidx * DVE_SH) // 192 != ((pidx + 1) * DVE_SH) // 192:
                                # DVE bit-trick path
                                nc.vector.tensor_scalar(
                                    out=scr_i16[:, 0:fd],
                                    in0=s[:, 0:fd],
                                    scalar1=SC16,
                                    scalar2=SB16,
                                    op0=ALU.mult,
                                    op1=ALU.add,
                                )
                                hf = fd // 2
                                nc.vector.tensor_tensor(
                                    out=tr1[:, 0 : hf // 1],
                                    in0=scr_i16[:, 0:hf].bitcast(bf16),
                                    in1=scr_i16[:, hf:fd].bitcast(bf16),
                                    op=ALU.add,
                                )
                                qf = hf // 2
                                nc.vector.tensor_tensor(
                                    out=tr2[:, 0:qf],
                                    in0=tr1[:, 0:qf],
                                    in1=tr1[:, qf:hf],
                                    op=ALU.add,
                                )
                                nc.vector.tensor_scalar(
                                    out=tr3[:, 0:qf],
                                    in0=tr2[:, 0:qf],
                                    scalar1=1.0,
                                    scalar2=None,
                                    op0=ALU.mult,
                                    op1=ALU.add,
                                    accum_out=lse_acc[:, col : col + 1],
                                )
                            else:
                                nc.scalar.activation(
                                    out=s[:, 0:fd],
                                    in_=s[:, 0:fd],
                                    func=AF.Exp,
                                    scale=SCALE,
                                    accum_out=lse_acc[:, col : col + 1],
                                )
                            pidx += 1
                # flush leftover rides + drains (post-stream)
                while drainq:
                    drainq.pop(0)[1]()
                while state["ride"] < len(rides):
                    _, mm_fn, drain_fn = rides[state["ride"]]
                    if mm_fn is not None:
                        mm_fn()
                    if drain_fn is not None:
                        drain_fn()
                    state["ride"] += 1

                # ---- tail: Ln + rank-1 + projection ------------------------
                # lse: sum the 3 per-piece accumulator cols -> [128, 64]
                la = lse_acc[:].rearrange("q (p three) -> q p three", three=3)
                nc.vector.tensor_add(lse_sum[:], la[:, :, 0], la[:, :, 1])
                nc.vector.tensor_add(lse_sum[:], lse_sum[:], la[:, :, 2])
                nc.scalar.activation(out=lse_ln[:], in_=lse_sum[:], func=AF.Ln)
                lse_tm = const.tile([128, 64], bf16, tag="lse_tm")
                nc.vector.tensor_copy(
                    lse_tm[:],
                    lse_ln[:].rearrange("q (h t) -> q t h", t=QT),
                )
                for t in range(QT):
                    sl = slot[t % 2]
                    ps_bf = sl[0:8, 0:64].bitcast(bf16)
                    nc.tensor.transpose(ps_bf, lse_tm[:, t * 8 : (t + 1) * 8], ident_bf[:])
                    nc.vector.tensor_copy(lnST[:, t * 128 : (t + 1) * 128], ps_bf)
                    yps = sl[:, 512:1024]
                    for c in range(4):
                        nc.tensor.matmul(
                            yps,
                            lhsT=qT[c][:, t * 128 : (t + 1) * 128],
                            rhs=KVW[c][:],
                            start=(c == 0),
                            stop=False,
                        )
                    nc.tensor.matmul(
                        yps,
                        lhsT=lnST[:, t * 128 : (t + 1) * 128],
                        rhs=W8_sb[:],
                        start=False,
                        stop=True,
                    )
                    ysb = dout.tile([128, D], f32, name="ysb", tag="ysb")
                    nc.vector.tensor_add(ysb[:], yps, xb[t][:])
                    nc.sync.dma_start(out=out_d[t * 128 : (t + 1) * 128, :], in_=ysb[:])

    nc.compile()
    return nc


def get_graph():
    if "nc" not in _GRAPH_CACHE:
        _GRAPH_CACHE["nc"] = _build_graph()
    return _GRAPH_CACHE["nc"]


def make_in_maps(x, w_qkv, w_out, b_out):
    import ml_dtypes

    x = np.ascontiguousarray(x, dtype=np.float32)
    w_qkv = np.ascontiguousarray(w_qkv, dtype=np.float32)
    w_out = np.ascontiguousarray(w_out, dtype=np.float32)
    b_out = np.ascontiguousarray(b_out, dtype=np.float32)
    x_bf = x.astype(ml_dtypes.bfloat16)
    w_qkv_bf = w_qkv.astype(ml_dtypes.bfloat16)
    w_out_bf = w_out.astype(ml_dtypes.bfloat16)
    in_maps = []
    for i in range(8):
        b, q = divmod(i, 4)
        in_maps.append(
            {
                # keys are permutation-invariant for lse/kv/G; roll so this
                # core's own query rows sit at rows 0:NQ
                "x_bf": np.ascontiguousarray(np.roll(x_bf[b], -q * NQ, axis=0)),
                "xq": np.ascontiguousarray(x[b, q * NQ : (q + 1) * NQ]),
                "w_qkv_bf": w_qkv_bf,
                "w_out_bf": w_out_bf,
                "b_out": b_out,
            }
        )
    return in_maps


def kernel(x, w_qkv, w_out, b_out):
    from concourse.bass_utils import run_bass_kernel_spmd

    nc = get_graph()
    in_maps = make_in_maps(x, w_qkv, w_out, b_out)
    res = run_bass_kernel_spmd(nc, in_maps, core_ids=list(range(8)))
    out = np.empty((B, N, D), np.float32)
    for i in range(8):
        b, q = divmod(i, 4)
        out[b, q * NQ : (q + 1) * NQ] = res.results[i]["out"]
    return out
